# revision 31
# baseline (speedup 1.0000x reference)
"""Trainium2 Bass kernel for DigitCapsules dynamic routing (v4).

Problem: u [256, 2048, 8] f32, W [1, 2048, 10, 16, 8] f32
  u_hat = einsum('pcoi,bpi->bpco', W[0], u)
  3 routing iterations (softmax over c, weighted sum over p, squash,
  agreement update) -> v [256, 10, 16] f32.

Strategy (8 cores, data-parallel over batch, 32 batch elems per core):
  - Partition layout: slabs of 16 p-values; SBUF partition index =
    (p_local * 8 + b_member); the PE contraction runs over
    K = (p_local 16, i 8) = 128 via a block-diagonal stationary u_bd
    (host-built; the zeros cost nothing at matmul time).
  - W columns are (o,c)-ordered host-side, so PSUM arrives as
    [slab, o, c] and evacuation is a straight (non-transposing) copy.
  - u_hat is materialized as FOUR quarter tiles per group ([128, 32, O,
    C] bf16, c innermost) so routing starts as soon as the first
    quarter lands instead of waiting for all 128 slabs.
  - Iteration 1 needs no u_hat read: s1 = 0.1 * sum_p u_hat from a
    dense-u stationary matmul accumulated over all slabs.
  - Iterations 2,3: G via DVE mul + add-tree over o; softmax over c via
    ACT exp + DVE reduce; weighted s-sum via PE matmuls with TWO
    block-diagonal ones stationaries that accumulate the group pair
    into one [16, 320] PSUM tile (squash is pair-batched).
  - Squash: one-Newton rsqrt (bit-trick seed), ~15 DVE ops per pair.
  - v never roundtrips through DRAM: V_rep replication and the v1
    accumulator are SBUF->SBUF block-copy DMAs.
  - The first pair's FINAL squash is emitted after the second pair's
    it1 routing so it fills the second pair's squash/vrep stall.
"""

import numpy as np
import ml_dtypes

bf16 = ml_dtypes.bfloat16

# Problem constants (fixed by the problem spec; do not read spec.json here)
B, P, C, O, IN = 256, 2048, 10, 16, 8
NCORES = 8
B_LOC = B // NCORES          # 32 batch elems per core
BT = 8                       # batch elems per group (one octet)
NGROUP = B_LOC // BT         # 4 groups per core
PSLAB = 16                   # p-values per slab
NSLAB = P // PSLAB           # 128 slabs
CO = C * O                   # 160
ROUTING_ITERS = 3
EPS = 1e-9

CHB = 8     # slabs per u_bd DMA chunk
EV = 3      # slabs per PSUM evacuation batch (1 bank per tile)
SMM = 2     # slabs per s-step matmul (N = SMM*CO = 320 <= 512)
CHS = 32    # slabs per routing compute chunk (= uhat quarter size)


def _host_prep(u_core, W0, nslab=NSLAB, ngroup=NGROUP):
    """Build host-side reordered (k-major, contiguous-DMA) arrays."""
    # w_k[p*8+i, s, o*10+c] = W0[16s+p, c, o, i]  ((o,c)-ordered columns)
    w = W0.reshape(nslab, PSLAB, C, O, IN)
    w_k = np.ascontiguousarray(
        w.transpose(1, 4, 0, 3, 2).reshape(PSLAB * IN, nslab, CO)
    ).astype(bf16)

    # x[g, b, s, p, i] = u_core[g*8 + b, 16s+p, i]
    x = u_core.reshape(ngroup, BT, nslab, PSLAB, IN)

    # ubd_k[g, ci, p*8+i, s_in_chunk, p'*8+b] = x[g,b,ci*chb+s,p,i]*(p==p')
    # -- chunk-major so every u_bd DMA is one fully contiguous read
    xt = x.transpose(0, 3, 4, 2, 1)  # [g, p, i, s, b]
    ubd_k = np.zeros((ngroup, PSLAB, IN, nslab, PSLAB, BT), dtype=bf16)
    for p in range(PSLAB):
        ubd_k[:, p, :, :, p, :] = xt[:, p]
    chb = min(CHB, nslab)
    ubd_k = np.ascontiguousarray(
        ubd_k.reshape(ngroup, PSLAB * IN, nslab // chb, chb, PSLAB * BT)
        .transpose(0, 2, 1, 3, 4))

    # ut_k[p*8+i, s, g*8+b] = x[g,b,s,p,i] -- one dense stationary for the
    # s1 matmul covering ALL groups (M = ngroup*BT)
    ut_k = np.ascontiguousarray(
        x.transpose(3, 4, 2, 0, 1).reshape(PSLAB * IN, nslab, ngroup * BT)
    ).astype(bf16)

    # ones2[j, p*8+b, j'*8+b'] = (b == b') & (j == j') -- stationaries for
    # the s-reduction; group j of a pair fills output rows j*8..j*8+8 while
    # contributing zeros to the other half (PSUM-accumulated jointly).
    ones2 = np.zeros((2, PSLAB * BT, 2 * BT), dtype=bf16)
    for j in range(2):
        for p in range(PSLAB):
            for b in range(BT):
                ones2[j, p * BT + b, j * BT + b] = 1.0
    return {
        "w_k": w_k,
        "ubd_k": ubd_k,
        "ut_k": ut_k,
        "ones2": ones2,
    }


def build(nc, tc, ctx, nslab=NSLAB, ngroup=NGROUP):
    """Emit the kernel IR. Parameterized slab/group counts for small tests."""
    import concourse.bass as bass
    from concourse import mybir

    f32 = mybir.dt.float32
    i32 = mybir.dt.int32
    bf = mybir.dt.bfloat16
    Alu = mybir.AluOpType
    Act = mybir.ActivationFunctionType
    Ax = mybir.AxisListType

    b_loc = ngroup * BT
    chb = min(CHB, nslab)
    ev = min(EV, nslab)
    smm = min(SMM, nslab)
    chs = min(CHS, nslab)
    nq = (nslab + chs - 1) // chs   # quarters per group

    # ---- DRAM parameters ----
    w_dram = nc.dram_tensor(
        "w_k", [PSLAB * IN, nslab, CO], bf, kind="ExternalInput").ap()
    ubd_dram = nc.dram_tensor(
        "ubd_k", [ngroup, nslab // min(CHB, nslab), PSLAB * IN,
                  min(CHB, nslab), PSLAB * BT], bf,
        kind="ExternalInput").ap()
    ut_dram = nc.dram_tensor(
        "ut_k", [PSLAB * IN, nslab, ngroup * BT], bf,
        kind="ExternalInput").ap()
    ones_dram = nc.dram_tensor(
        "ones2", [2, PSLAB * BT, 2 * BT], bf, kind="ExternalInput").ap()
    vout_dram = nc.dram_tensor(
        "v_out", [b_loc, CO], f32, kind="ExternalOutput").ap()

    # ---- pools ----
    consts = ctx.enter_context(tc.tile_pool(name="consts", bufs=1))
    ubdpool = ctx.enter_context(tc.tile_pool(name="ubdpool", bufs=2))
    utpool = ctx.enter_context(tc.tile_pool(name="utpool", bufs=1))
    uhatpool = ctx.enter_context(tc.tile_pool(name="uhat", bufs=2 * nq))
    psum = ctx.enter_context(tc.tile_pool(name="psum", bufs=4, space="PSUM"))
    psum_acc = ctx.enter_context(
        tc.tile_pool(name="psum_acc", bufs=2, space="PSUM"))
    small = ctx.enter_context(tc.tile_pool(name="small", bufs=2))
    state = ctx.enter_context(tc.tile_pool(name="state", bufs=2))
    tmp = ctx.enter_context(tc.tile_pool(name="tmp", bufs=2))

    ones_sb = consts.tile([PSLAB * BT, 2, 2 * BT], bf)
    for j in range(2):
        nc.sync.dma_start(out=ones_sb[:, j, :], in_=ones_dram[j])
    magic = consts.tile([128, 1], i32)
    nc.gpsimd.memset(magic[:], 0x5F3759DF)

    # v1 accumulator per pair [16, O*C] (o,c)-major bf16
    npair = (ngroup + 1) // 2
    v1k = [consts.tile([2 * BT, O * C], bf, name=f"v1k{j}")
           for j in range(npair)]

    # resident W: whole tensor, contiguous quarters across the queues
    wall = consts.tile([PSLAB * IN, nslab, CO], bf)
    h = max(1, nslab // 4)
    engs = [nc.scalar, nc.sync, nc.gpsimd, nc.scalar]
    for jj, j in enumerate(range(0, nslab, h)):
        engs[jj % 4].dma_start(
            out=wall[:, j:j + h, :], in_=w_dram[:, j:j + h, :])

    def bcast_ap(ap, insert_pos, size):
        """Insert a stride-0 dim of `size` at free-dim position insert_pos."""
        new = list(ap.ap)
        new.insert(insert_pos, [0, size])
        return bass.AP(tensor=ap.tensor, offset=ap.offset, ap=new)

    def squash(s_sb, n, it, vk=None):
        """s_sb: [n, CO] f32, (o,c)-major. it < last: returns v_bf [n, O*C]
        bf16 (o,c) with accumulated V (vk + v) when vk given; else final:
        returns v_sb f32 (o,c).  factor = nrm/((1+nrm)*sqrt(nrm+eps)),
        rsqrt via bit-trick seed + one Newton step (~0.2% rel err)."""
        s3 = s_sb[:].rearrange("n (o c) -> n c o", o=O)
        sq = small.tile([n, CO], f32, tag="sqsq", bufs=1)
        nc.vector.tensor_mul(sq[:].rearrange("n (o c) -> n c o", o=O), s3, s3)
        nrm = small.tile([n, C], f32, tag="nrm")
        nc.vector.tensor_reduce(
            out=nrm[:], in_=sq[:].rearrange("n (o c) -> n c o", o=O),
            axis=Ax.X, op=Alu.add)
        d1 = small.tile([n, C], f32, tag="d1")
        nc.vector.tensor_scalar_add(d1[:], nrm[:], 1.0)
        r1 = small.tile([n, C], f32, tag="r1")
        nc.vector.reciprocal(r1[:], d1[:])
        se = small.tile([n, C], f32, tag="se")
        nc.vector.tensor_scalar_add(se[:], nrm[:], EPS)
        sh = small.tile([n, C], i32, tag="sh")
        nc.vector.tensor_scalar(
            out=sh[:], in0=se[:].bitcast(i32), scalar1=1, scalar2=None,
            op0=Alu.logical_shift_right)
        y0 = small.tile([n, C], i32, tag="y0")
        nc.vector.tensor_tensor(
            out=y0[:], in0=bcast_ap(magic[0:n, :], 1, C), in1=sh[:],
            op=Alu.subtract)
        y = y0[:].bitcast(f32)
        aa = small.tile([n, C], f32, tag="na")
        nc.vector.tensor_tensor(out=aa[:], in0=y, in1=y, op=Alu.mult)
        nc.vector.tensor_tensor(out=aa[:], in0=aa[:], in1=se[:],
                                op=Alu.mult)
        nc.vector.tensor_scalar(
            out=aa[:], in0=aa[:], scalar1=-0.5, scalar2=1.5,
            op0=Alu.mult, op1=Alu.add)
        yn = small.tile([n, C], f32, tag="ny")
        nc.vector.tensor_tensor(out=yn[:], in0=y, in1=aa[:], op=Alu.mult)
        f1 = small.tile([n, C], f32, tag="f1")
        nc.vector.tensor_mul(f1[:], nrm[:], r1[:])
        fac = small.tile([n, C], f32, tag="fac")
        nc.vector.tensor_mul(fac[:], f1[:], yn[:])
        v_sb = small.tile([n, CO], f32, tag="v_sb")
        nc.vector.tensor_tensor(
            out=v_sb[:].rearrange("n (o c) -> n c o", o=O),
            in0=s3, in1=bcast_ap(fac[:], 2, O), op=Alu.mult)
        if it == ROUTING_ITERS - 1:
            return v_sb
        v_bf = small.tile([n, O * C], bf, tag="v_bf")
        if vk is not None:
            # accumulated V = v1 + v2 so the next iteration's logits come
            # out of one linear G pass
            nc.vector.tensor_tensor(
                out=v_bf[:], in0=v_sb[:], in1=vk[0:n, :], op=Alu.add)
        else:
            nc.vector.tensor_copy(out=v_bf[:], in_=v_sb[:])
        return v_bf

    def load_vrep(V_rep, v_bf, r0):
        """Replicate v rows [r0:r0+8] across the 16 p-positions (16 small
        SBUF->SBUF block-copy DMAs; scalar queue only, so they never sit
        behind the bulk u_bd traffic on sync/gpsimd)."""
        for p in range(PSLAB):
            eng = nc.scalar
            eng.dma_start(
                out=V_rep[p * BT:(p + 1) * BT, :, :]
                    .rearrange("n o c -> n (o c)"),
                in_=v_bf[r0:r0 + BT, :])

    def emit_s1():
        # ------- s1 sweep: one accumulation for ALL groups (M=32) -------
        ut_res = utpool.tile([PSLAB * IN, nslab, ngroup * BT], bf,
                             tag="utres", name="ut_res")
        nc.sync.dma_start(out=ut_res[:], in_=ut_dram)
        s1_ps = psum_acc.tile([ngroup * BT, CO], f32, tag="s1ps",
                              name="s1ps")
        for s in range(nslab):
            nc.tensor.matmul(
                out=s1_ps[:], lhsT=ut_res[:, s, :], rhs=wall[:, s, :],
                start=(s == 0), stop=(s == nslab - 1))
        s1_sb = consts.tile([ngroup * BT, CO], f32)
        nc.scalar.mul(s1_sb[:], s1_ps[:], 1.0 / C)
        v_bf1 = squash(s1_sb, ngroup * BT, 0)
        for j in range(npair):
            r = min(2 * BT, b_loc - j * 2 * BT)
            nc.scalar.dma_start(out=v1k[j][0:r, :],
                                in_=v_bf1[j * 2 * BT:j * 2 * BT + r, :])
        return v_bf1

    def phase_a(g, q_from=0, q_to=None):
        """u_hat materialization for quarters [q_from, q_to); returns the
        quarter tiles."""
        if q_to is None:
            q_to = nq
        quarters = []
        ubs = {}

        def get_ub(ci):
            if ci not in ubs:
                ub = ubdpool.tile([PSLAB * IN, chb, PSLAB * BT], bf,
                                  tag="ubd", name="ubd")
                eng = nc.gpsimd if ci % 2 else nc.sync
                eng.dma_start(out=ub[:], in_=ubd_dram[g, ci])
                ubs[ci] = ub
            return ubs[ci]

        for q0 in range(q_from * chs, min(q_to * chs, nslab), chs):
            qn = min(chs, nslab - q0)
            uq = uhatpool.tile([128, chs, O, C], bf, tag="uhat",
                               name=f"uhat{g}_{q0}")
            quarters.append(uq)
            s0 = q0
            while s0 < q0 + qn:
                nb = min(ev, q0 + qn - s0)
                ps = psum.tile([128, ev, CO], f32, tag="ups", name="ups")
                get_ub(s0 // chb)
                get_ub((s0 + nb - 1) // chb)
                for qq in range(nb):
                    sl = s0 + qq
                    ub = ubs[sl // chb]
                    nc.tensor.matmul(
                        out=ps[:, qq, :], lhsT=ub[:, sl % chb, :],
                        rhs=wall[:, sl, :], start=True, stop=True)
                # (o,c)-ordered W columns -> straight PSUM->SBUF copy
                nc.scalar.copy(
                    uq[:, s0 - q0:s0 - q0 + nb, :, :]
                        .rearrange("p s o c -> p (s o c)"),
                    ps[:, 0:nb, :].rearrange("p s x -> p (s x)"))
                s0 += nb
        return quarters

    def route_core(g, uq, V_rep, s_ps, j, jlast):
        """One group's G/softmax/premul/s-matmul for one iteration.
        uq: list of uhat quarter tiles. The s-matmuls accumulate into the
        pair-shared s_ps [16, smm*CO] via the ones2[j] stationary."""
        # b-state scratch (bf16: G magnitudes are <<1)
        bst = state.tile([128, nslab, C], bf, tag="bst", name="bst")
        nchunk = nq

        # ---- G-step: bst = sum_o uhat * V_rep ----
        for ch in range(nchunk):
            sl = slice(ch * chs, (ch + 1) * chs)
            u4 = uq[ch]
            t2 = tmp.tile([128, chs, O, C], bf, tag="t2", bufs=3)
            nc.vector.tensor_tensor(
                out=t2[:], in0=u4[:],
                in1=bcast_ap(V_rep[:], 1, chs), op=Alu.mult)
            r1 = tmp.tile([128, chs, O // 2, C], bf, tag="r1t")
            nc.vector.tensor_tensor(
                out=r1[:], in0=t2[:, :, 0:O // 2, :],
                in1=t2[:, :, O // 2:O, :], op=Alu.add)
            r2 = tmp.tile([128, chs, O // 4, C], bf, tag="r2t")
            nc.vector.tensor_tensor(
                out=r2[:], in0=r1[:, :, 0:O // 4, :],
                in1=r1[:, :, O // 4:O // 2, :], op=Alu.add)
            r3 = tmp.tile([128, chs, 2, C], bf, tag="r3t")
            nc.vector.tensor_tensor(
                out=r3[:], in0=r2[:, :, 0:2, :],
                in1=r2[:, :, 2:4, :], op=Alu.add)
            nc.vector.tensor_tensor(
                out=bst[:, sl, :], in0=r3[:, :, 0, :],
                in1=r3[:, :, 1, :], op=Alu.add)
        # ---- softmax over c (2 coarse chunks to cut instr count) ----
        expt = tmp.tile([128, nslab, C], bf, tag="expt", bufs=2)
        Z = tmp.tile([128, nslab], f32, tag="Z")
        rz = tmp.tile([128, nslab], f32, tag="rz")
        cw = tmp.tile([128, nslab, C], bf, tag="cw", bufs=2)
        nsoft = min(2, nchunk)
        sft = nslab // nsoft
        for hf in range(nsoft):
            hs = slice(hf * sft, (hf + 1) * sft)
            nc.scalar.activation(expt[:, hs, :], bst[:, hs, :], Act.Exp)
            nc.vector.tensor_reduce(
                out=Z[:, hs], in_=expt[:, hs, :], axis=Ax.X, op=Alu.add)
            nc.vector.reciprocal(rz[:, hs], Z[:, hs])
            nc.vector.tensor_tensor(
                out=cw[:, hs, :], in0=expt[:, hs, :],
                in1=bcast_ap(rz[:, hs], 2, C), op=Alu.mult)
        # ---- s-step: premul + PE block-diag ones reduction ----
        for ch in range(nchunk):
            sl = slice(ch * chs, (ch + 1) * chs)
            u4 = uq[ch]
            t1 = tmp.tile([128, chs, O, C], bf, tag="t2", bufs=3)
            nc.vector.tensor_tensor(
                out=t1[:], in0=u4[:],
                in1=bcast_ap(cw[:, sl, :], 2, O), op=Alu.mult)
            for k in range(chs // smm):
                s_idx = ch * chs + k * smm
                nc.tensor.matmul(
                    out=s_ps[:], lhsT=ones_sb[:, j, :],
                    rhs=t1[:, k * smm:(k + 1) * smm, :, :],
                    start=(j == 0 and s_idx == 0),
                    stop=(j == jlast and s_idx == nslab - smm))

    def collect_s(s_ps, nrow):
        """Sum the smm slab-positions -> s_pair [nrow, CO] ((o,c) order)."""
        s_pair = small.tile([nrow, CO], f32, tag="s_pair", bufs=2)
        if smm == 2:
            s_rw = small.tile([nrow, 2 * CO], f32, tag="s_rw")
            nc.scalar.copy(s_rw[:], s_ps[0:nrow, :])
            nc.vector.tensor_tensor(
                out=s_pair[:], in0=s_rw[:, 0:CO], in1=s_rw[:, CO:2 * CO],
                op=Alu.add)
        else:
            nc.scalar.copy(s_pair[:], s_ps[0:nrow, 0:CO])
        return s_pair

    def emit_final(s_ps, nrow, row0):
        """Deferred final squash + output DMA for a pair."""
        s_pair = collect_s(s_ps, nrow)
        v_sb = squash(s_pair, nrow, ROUTING_ITERS - 1)
        nc.sync.dma_start(
            out=vout_dram[row0:row0 + nrow, :], in_=v_sb[:])

    # Group 0's first quarter is emitted BEFORE the s1 sweep so its PE/DMA
    # work overlaps the sweep and routing can start right after s1's squash.
    ua0 = phase_a(0, 0, 1)
    v_bf1 = emit_s1()

    # Pair loop; the previous pair's FINAL squash is emitted after this
    # pair's it1 routing so it fills the squash/vrep stall.
    pending_final = None
    for j in range(npair):
        ga, gb = 2 * j, 2 * j + 1
        two = gb < ngroup
        nrow = 2 * BT if two else BT
        jlast = 1 if two else 0
        ua = (ua0 + phase_a(ga, 1)) if j == 0 else phase_a(ga)
        ub = phase_a(gb) if two else None
        Va = state.tile([128, O, C], bf, tag="vrep", name="vrep", bufs=4)
        load_vrep(Va, v_bf1, ga * BT)
        if two:
            Vb = state.tile([128, O, C], bf, tag="vrep", name="vrepb",
                            bufs=4)
            load_vrep(Vb, v_bf1, gb * BT)
        for it in range(1, ROUTING_ITERS):
            s_ps = psum_acc.tile([2 * BT, smm * CO], f32, tag="sps",
                                 name="sps")
            route_core(ga, ua, Va, s_ps, 0, jlast)
            if two:
                route_core(gb, ub, Vb, s_ps, 1, jlast)
            if pending_final is not None:
                emit_final(*pending_final)
                pending_final = None
            last = (it == ROUTING_ITERS - 1)
            if last:
                pending_final = (s_ps, nrow, ga * BT)
            else:
                s_pair = collect_s(s_ps, nrow)
                v_bf = squash(s_pair, nrow, it, vk=v1k[j])
                Va = state.tile([128, O, C], bf, tag="vrep", name="vrep2",
                                bufs=4)
                load_vrep(Va, v_bf, 0)
                if two:
                    Vb = state.tile([128, O, C], bf, tag="vrep",
                                    name="vrep2b", bufs=4)
                    load_vrep(Vb, v_bf, BT)
    if pending_final is not None:
        emit_final(*pending_final)


def make_inputs_per_core(u, W):
    """Full inputs -> list of 8 in_maps."""
    W0 = np.asarray(W, dtype=np.float32)[0]
    u = np.asarray(u, dtype=np.float32)
    in_maps = []
    for c in range(NCORES):
        u_core = u[c * B_LOC:(c + 1) * B_LOC]
        in_maps.append(_host_prep(u_core, W0))
    return in_maps


def numpy_model(u_core, W0):
    """f32 numpy model of the routing (for small-scale checks)."""
    u_hat = np.einsum('pcoi,bpi->bpco', W0, u_core)
    b = np.zeros(u_hat.shape[:3], dtype=np.float32)
    v = None
    for _ in range(ROUTING_ITERS):
        e = np.exp(b - b.max(axis=2, keepdims=True))
        c = e / e.sum(axis=2, keepdims=True)
        s = np.einsum('bpc,bpco->bco', c, u_hat)
        sq = (s * s).sum(-1, keepdims=True)
        v = (sq / (1 + sq)) * s / np.sqrt(sq + EPS)
        b = b + np.einsum('bpco,bco->bpc', u_hat, v)
    return v


_COMPILED = {}


def _get_compiled():
    if "nc" in _COMPILED:
        return _COMPILED["nc"]
    from contextlib import ExitStack
    import concourse.tile as tile
    from concourse import bacc

    nc = bacc.Bacc("TRN2", target_bir_lowering=False, debug=False,
                   num_devices=NCORES)
    with tile.TileContext(nc) as tc:
        with ExitStack() as ctx:
            build(nc, tc, ctx)
    nc.compile()
    _COMPILED["nc"] = nc
    return nc


def kernel(u, W):
    """Full-input entry point: u [256,2048,8] f32, W [1,2048,10,16,8] f32
    -> v [256, 10, 16] f32."""
    from concourse.bass_utils import run_bass_kernel_spmd

    nc = _get_compiled()
    in_maps = make_inputs_per_core(u, W)
    res = run_bass_kernel_spmd(nc, in_maps, core_ids=list(range(NCORES)))
    outs = [res.results[c]["v_out"] for c in range(NCORES)]
    # v_out rows are (o,c)-major -> [B, O, C] -> transpose to [B, C, O]
    v = np.concatenate(outs, axis=0).reshape(B, O, C)
    return np.ascontiguousarray(v.transpose(0, 2, 1)).astype(np.float32)


# revision 32
# speedup vs baseline: 1.0679x; 1.0679x over previous
"""Trainium2 Bass kernel for DigitCapsules dynamic routing (v4).

Problem: u [256, 2048, 8] f32, W [1, 2048, 10, 16, 8] f32
  u_hat = einsum('pcoi,bpi->bpco', W[0], u)
  3 routing iterations (softmax over c, weighted sum over p, squash,
  agreement update) -> v [256, 10, 16] f32.

Strategy (8 cores, data-parallel over batch, 32 batch elems per core):
  - Partition layout: slabs of 16 p-values; SBUF partition index =
    (p_local * 8 + b_member); the PE contraction runs over
    K = (p_local 16, i 8) = 128 via a block-diagonal stationary u_bd
    (host-built; the zeros cost nothing at matmul time).
  - W columns are (o,c)-ordered host-side, so PSUM arrives as
    [slab, o, c] and evacuation is a straight (non-transposing) copy.
  - u_hat is materialized as FOUR quarter tiles per group ([128, 32, O,
    C] bf16, c innermost) so routing starts as soon as the first
    quarter lands instead of waiting for all 128 slabs.
  - Iteration 1 needs no u_hat read: s1 = 0.1 * sum_p u_hat from a
    dense-u stationary matmul accumulated over all slabs.
  - Iterations 2,3: G via DVE mul + add-tree over o; softmax over c via
    ACT exp + DVE reduce; weighted s-sum via PE matmuls with TWO
    block-diagonal ones stationaries that accumulate the group pair
    into one [16, 320] PSUM tile (squash is pair-batched).
  - Squash: one-Newton rsqrt (bit-trick seed), ~15 DVE ops per pair.
  - v never roundtrips through DRAM: V_rep replication and the v1
    accumulator are SBUF->SBUF block-copy DMAs.
  - The first pair's FINAL squash is emitted after the second pair's
    it1 routing so it fills the second pair's squash/vrep stall.
"""

import numpy as np
import ml_dtypes

bf16 = ml_dtypes.bfloat16

# Problem constants (fixed by the problem spec; do not read spec.json here)
B, P, C, O, IN = 256, 2048, 10, 16, 8
NCORES = 8
B_LOC = B // NCORES          # 32 batch elems per core
BT = 8                       # batch elems per group (one octet)
NGROUP = B_LOC // BT         # 4 groups per core
PSLAB = 16                   # p-values per slab
NSLAB = P // PSLAB           # 128 slabs
CO = C * O                   # 160
ROUTING_ITERS = 3
EPS = 1e-9

CHB = 8     # slabs per u_bd DMA chunk
EV = 3      # slabs per PSUM evacuation batch (1 bank per tile)
SMM = 2     # slabs per s-step matmul (N = SMM*CO = 320 <= 512)
CHS = 32    # slabs per routing compute chunk (= uhat quarter size)


def _host_prep(u_core, W0, nslab=NSLAB, ngroup=NGROUP):
    """Build host-side reordered (k-major, contiguous-DMA) arrays."""
    # w_k[p*8+i, s, o*10+c] = W0[16s+p, c, o, i]  ((o,c)-ordered columns)
    w = W0.reshape(nslab, PSLAB, C, O, IN)
    w_k = np.ascontiguousarray(
        w.transpose(1, 4, 0, 3, 2).reshape(PSLAB * IN, nslab, CO)
    ).astype(bf16)

    # x[g, b, s, p, i] = u_core[g*8 + b, 16s+p, i]
    x = u_core.reshape(ngroup, BT, nslab, PSLAB, IN)

    # ubd_k[g, ci, p*8+i, s_in_chunk, p'*8+b] = x[g,b,ci*chb+s,p,i]*(p==p')
    # -- chunk-major so every u_bd DMA is one fully contiguous read
    xt = x.transpose(0, 3, 4, 2, 1)  # [g, p, i, s, b]
    ubd_k = np.zeros((ngroup, PSLAB, IN, nslab, PSLAB, BT), dtype=bf16)
    for p in range(PSLAB):
        ubd_k[:, p, :, :, p, :] = xt[:, p]
    chb = min(CHB, nslab)
    ubd_k = np.ascontiguousarray(
        ubd_k.reshape(ngroup, PSLAB * IN, nslab // chb, chb, PSLAB * BT)
        .transpose(0, 2, 1, 3, 4))

    # ut_k[p*8+i, s, g*8+b] = x[g,b,s,p,i] -- one dense stationary for the
    # s1 matmul covering ALL groups (M = ngroup*BT)
    ut_k = np.ascontiguousarray(
        x.transpose(3, 4, 2, 0, 1).reshape(PSLAB * IN, nslab, ngroup * BT)
    ).astype(bf16)

    # ones2[j, p*8+b, j'*8+b'] = (b == b') & (j == j') -- stationaries for
    # the s-reduction; group j of a pair fills output rows j*8..j*8+8 while
    # contributing zeros to the other half (PSUM-accumulated jointly).
    ones2 = np.zeros((2, PSLAB * BT, 2 * BT), dtype=bf16)
    for j in range(2):
        for p in range(PSLAB):
            for b in range(BT):
                ones2[j, p * BT + b, j * BT + b] = 1.0
    return {
        "w_k": w_k,
        "ubd_k": ubd_k,
        "ut_k": ut_k,
        "ones2": ones2,
    }


def build(nc, tc, ctx, nslab=NSLAB, ngroup=NGROUP):
    """Emit the kernel IR. Parameterized slab/group counts for small tests."""
    import concourse.bass as bass
    from concourse import mybir

    f32 = mybir.dt.float32
    i32 = mybir.dt.int32
    bf = mybir.dt.bfloat16
    Alu = mybir.AluOpType
    Act = mybir.ActivationFunctionType
    Ax = mybir.AxisListType

    b_loc = ngroup * BT
    chb = min(CHB, nslab)
    ev = min(EV, nslab)
    smm = min(SMM, nslab)
    chs = min(CHS, nslab)
    nq = (nslab + chs - 1) // chs   # quarters per group

    # ---- DRAM parameters ----
    w_dram = nc.dram_tensor(
        "w_k", [PSLAB * IN, nslab, CO], bf, kind="ExternalInput").ap()
    ubd_dram = nc.dram_tensor(
        "ubd_k", [ngroup, nslab // min(CHB, nslab), PSLAB * IN,
                  min(CHB, nslab), PSLAB * BT], bf,
        kind="ExternalInput").ap()
    ut_dram = nc.dram_tensor(
        "ut_k", [PSLAB * IN, nslab, ngroup * BT], bf,
        kind="ExternalInput").ap()
    ones_dram = nc.dram_tensor(
        "ones2", [2, PSLAB * BT, 2 * BT], bf, kind="ExternalInput").ap()
    vout_dram = nc.dram_tensor(
        "v_out", [b_loc, CO], f32, kind="ExternalOutput").ap()

    # ---- pools ----
    consts = ctx.enter_context(tc.tile_pool(name="consts", bufs=1))
    ubdpool = ctx.enter_context(tc.tile_pool(name="ubdpool", bufs=2))
    utpool = ctx.enter_context(tc.tile_pool(name="utpool", bufs=1))
    uhatpool = ctx.enter_context(tc.tile_pool(name="uhat", bufs=2 * nq))
    psum = ctx.enter_context(tc.tile_pool(name="psum", bufs=4, space="PSUM"))
    psum_acc = ctx.enter_context(
        tc.tile_pool(name="psum_acc", bufs=2, space="PSUM"))
    small = ctx.enter_context(tc.tile_pool(name="small", bufs=2))
    state = ctx.enter_context(tc.tile_pool(name="state", bufs=2))
    tmp = ctx.enter_context(tc.tile_pool(name="tmp", bufs=2))

    ones_sb = consts.tile([PSLAB * BT, 2, 2 * BT], bf)
    for j in range(2):
        nc.sync.dma_start(out=ones_sb[:, j, :], in_=ones_dram[j])
    magic = consts.tile([128, 1], i32)
    nc.gpsimd.memset(magic[:], 0x5F3759DF)

    # v1 accumulator per pair [16, O*C] (o,c)-major bf16
    npair = (ngroup + 1) // 2
    v1k = [consts.tile([2 * BT, O * C], bf, name=f"v1k{j}")
           for j in range(npair)]

    # resident W: whole tensor, contiguous quarters across the queues
    wall = consts.tile([PSLAB * IN, nslab, CO], bf)
    h = max(1, nslab // 4)
    engs = [nc.scalar, nc.sync, nc.gpsimd, nc.scalar]
    for jj, j in enumerate(range(0, nslab, h)):
        engs[jj % 4].dma_start(
            out=wall[:, j:j + h, :], in_=w_dram[:, j:j + h, :])

    def bcast_ap(ap, insert_pos, size):
        """Insert a stride-0 dim of `size` at free-dim position insert_pos."""
        new = list(ap.ap)
        new.insert(insert_pos, [0, size])
        return bass.AP(tensor=ap.tensor, offset=ap.offset, ap=new)

    def squash(s_sb, n, it, vk=None):
        """s_sb: [n, CO] f32, (o,c)-major. it < last: returns v_bf [n, O*C]
        bf16 (o,c) with accumulated V (vk + v) when vk given; else final:
        returns v_sb f32 (o,c).  factor = nrm/((1+nrm)*sqrt(nrm+eps)),
        rsqrt via bit-trick seed + one Newton step (~0.2% rel err)."""
        s3 = s_sb[:].rearrange("n (o c) -> n c o", o=O)
        sq = small.tile([n, CO], f32, tag="sqsq", bufs=1)
        nc.vector.tensor_mul(sq[:].rearrange("n (o c) -> n c o", o=O), s3, s3)
        nrm = small.tile([n, C], f32, tag="nrm")
        nc.vector.tensor_reduce(
            out=nrm[:], in_=sq[:].rearrange("n (o c) -> n c o", o=O),
            axis=Ax.X, op=Alu.add)
        d1 = small.tile([n, C], f32, tag="d1")
        nc.vector.tensor_scalar_add(d1[:], nrm[:], 1.0)
        r1 = small.tile([n, C], f32, tag="r1")
        nc.vector.reciprocal(r1[:], d1[:])
        se = small.tile([n, C], f32, tag="se")
        nc.vector.tensor_scalar_add(se[:], nrm[:], EPS)
        sh = small.tile([n, C], i32, tag="sh")
        nc.vector.tensor_scalar(
            out=sh[:], in0=se[:].bitcast(i32), scalar1=1, scalar2=None,
            op0=Alu.logical_shift_right)
        y0 = small.tile([n, C], i32, tag="y0")
        nc.vector.tensor_tensor(
            out=y0[:], in0=bcast_ap(magic[0:n, :], 1, C), in1=sh[:],
            op=Alu.subtract)
        y = y0[:].bitcast(f32)
        aa = small.tile([n, C], f32, tag="na")
        nc.vector.tensor_tensor(out=aa[:], in0=y, in1=y, op=Alu.mult)
        nc.vector.tensor_tensor(out=aa[:], in0=aa[:], in1=se[:],
                                op=Alu.mult)
        nc.vector.tensor_scalar(
            out=aa[:], in0=aa[:], scalar1=-0.5, scalar2=1.5,
            op0=Alu.mult, op1=Alu.add)
        yn = small.tile([n, C], f32, tag="ny")
        nc.vector.tensor_tensor(out=yn[:], in0=y, in1=aa[:], op=Alu.mult)
        f1 = small.tile([n, C], f32, tag="f1")
        nc.vector.tensor_mul(f1[:], nrm[:], r1[:])
        fac = small.tile([n, C], f32, tag="fac")
        nc.vector.tensor_mul(fac[:], f1[:], yn[:])
        v_sb = small.tile([n, CO], f32, tag="v_sb")
        nc.vector.tensor_tensor(
            out=v_sb[:].rearrange("n (o c) -> n c o", o=O),
            in0=s3, in1=bcast_ap(fac[:], 2, O), op=Alu.mult)
        if it == ROUTING_ITERS - 1:
            return v_sb
        v_bf = small.tile([n, O * C], bf, tag="v_bf")
        if vk is not None:
            # accumulated V = v1 + v2 so the next iteration's logits come
            # out of one linear G pass
            nc.vector.tensor_tensor(
                out=v_bf[:], in0=v_sb[:], in1=vk[0:n, :], op=Alu.add)
        else:
            nc.vector.tensor_copy(out=v_bf[:], in_=v_sb[:])
        return v_bf

    def load_vrep(V_rep, v_bf, r0):
        """Replicate v rows [r0:r0+8] across the 16 p-positions (16 small
        SBUF->SBUF block-copy DMAs spread over the queues)."""
        for p in range(PSLAB):
            eng = (nc.sync, nc.gpsimd, nc.scalar)[p % 3]
            eng.dma_start(
                out=V_rep[p * BT:(p + 1) * BT, :, :]
                    .rearrange("n o c -> n (o c)"),
                in_=v_bf[r0:r0 + BT, :])

    def emit_s1():
        # ------- s1 sweep: one accumulation for ALL groups (M=32) -------
        ut_res = utpool.tile([PSLAB * IN, nslab, ngroup * BT], bf,
                             tag="utres", name="ut_res")
        nc.sync.dma_start(out=ut_res[:], in_=ut_dram)
        s1_ps = psum_acc.tile([ngroup * BT, CO], f32, tag="s1ps",
                              name="s1ps")
        for s in range(nslab):
            nc.tensor.matmul(
                out=s1_ps[:], lhsT=ut_res[:, s, :], rhs=wall[:, s, :],
                start=(s == 0), stop=(s == nslab - 1))
        s1_sb = consts.tile([ngroup * BT, CO], f32)
        nc.scalar.mul(s1_sb[:], s1_ps[:], 1.0 / C)
        v_bf1 = squash(s1_sb, ngroup * BT, 0)
        for j in range(npair):
            r = min(2 * BT, b_loc - j * 2 * BT)
            nc.scalar.dma_start(out=v1k[j][0:r, :],
                                in_=v_bf1[j * 2 * BT:j * 2 * BT + r, :])
        return v_bf1

    def phase_a(g, q_from=0, q_to=None):
        """u_hat materialization for quarters [q_from, q_to); returns the
        quarter tiles."""
        if q_to is None:
            q_to = nq
        quarters = []
        ubs = {}

        def get_ub(ci):
            if ci not in ubs:
                ub = ubdpool.tile([PSLAB * IN, chb, PSLAB * BT], bf,
                                  tag="ubd", name="ubd")
                eng = nc.gpsimd if ci % 2 else nc.sync
                eng.dma_start(out=ub[:], in_=ubd_dram[g, ci])
                ubs[ci] = ub
            return ubs[ci]

        for q0 in range(q_from * chs, min(q_to * chs, nslab), chs):
            qn = min(chs, nslab - q0)
            uq = uhatpool.tile([128, chs, O, C], bf, tag="uhat",
                               name=f"uhat{g}_{q0}")
            quarters.append(uq)
            s0 = q0
            while s0 < q0 + qn:
                nb = min(ev, q0 + qn - s0)
                ps = psum.tile([128, ev, CO], f32, tag="ups", name="ups")
                get_ub(s0 // chb)
                get_ub((s0 + nb - 1) // chb)
                for qq in range(nb):
                    sl = s0 + qq
                    ub = ubs[sl // chb]
                    nc.tensor.matmul(
                        out=ps[:, qq, :], lhsT=ub[:, sl % chb, :],
                        rhs=wall[:, sl, :], start=True, stop=True)
                # (o,c)-ordered W columns -> straight PSUM->SBUF copy
                nc.scalar.copy(
                    uq[:, s0 - q0:s0 - q0 + nb, :, :]
                        .rearrange("p s o c -> p (s o c)"),
                    ps[:, 0:nb, :].rearrange("p s x -> p (s x)"))
                s0 += nb
        return quarters

    def route_core(g, uq, V_rep, s_ps, j, jlast):
        """One group's G/softmax/premul/s-matmul for one iteration.
        uq: list of uhat quarter tiles. The s-matmuls accumulate into the
        pair-shared s_ps [16, smm*CO] via the ones2[j] stationary."""
        # b-state scratch (bf16: G magnitudes are <<1)
        bst = state.tile([128, nslab, C], bf, tag="bst", name="bst")
        nchunk = nq

        # ---- G-step: bst = sum_o uhat * V_rep ----
        for ch in range(nchunk):
            sl = slice(ch * chs, (ch + 1) * chs)
            u4 = uq[ch]
            t2 = tmp.tile([128, chs, O, C], bf, tag="t2", bufs=3)
            nc.vector.tensor_tensor(
                out=t2[:], in0=u4[:],
                in1=bcast_ap(V_rep[:], 1, chs), op=Alu.mult)
            r1 = tmp.tile([128, chs, O // 2, C], bf, tag="r1t")
            nc.vector.tensor_tensor(
                out=r1[:], in0=t2[:, :, 0:O // 2, :],
                in1=t2[:, :, O // 2:O, :], op=Alu.add)
            r2 = tmp.tile([128, chs, O // 4, C], bf, tag="r2t")
            nc.vector.tensor_tensor(
                out=r2[:], in0=r1[:, :, 0:O // 4, :],
                in1=r1[:, :, O // 4:O // 2, :], op=Alu.add)
            r3 = tmp.tile([128, chs, 2, C], bf, tag="r3t")
            nc.vector.tensor_tensor(
                out=r3[:], in0=r2[:, :, 0:2, :],
                in1=r2[:, :, 2:4, :], op=Alu.add)
            nc.vector.tensor_tensor(
                out=bst[:, sl, :], in0=r3[:, :, 0, :],
                in1=r3[:, :, 1, :], op=Alu.add)
        # ---- softmax over c (2 coarse chunks to cut instr count) ----
        expt = tmp.tile([128, nslab, C], bf, tag="expt", bufs=2)
        Z = tmp.tile([128, nslab], f32, tag="Z")
        rz = tmp.tile([128, nslab], f32, tag="rz")
        cw = tmp.tile([128, nslab, C], bf, tag="cw", bufs=2)
        nsoft = min(2, nchunk)
        sft = nslab // nsoft
        for hf in range(nsoft):
            hs = slice(hf * sft, (hf + 1) * sft)
            nc.scalar.activation(expt[:, hs, :], bst[:, hs, :], Act.Exp)
            nc.vector.tensor_reduce(
                out=Z[:, hs], in_=expt[:, hs, :], axis=Ax.X, op=Alu.add)
            nc.vector.reciprocal(rz[:, hs], Z[:, hs])
            nc.vector.tensor_tensor(
                out=cw[:, hs, :], in0=expt[:, hs, :],
                in1=bcast_ap(rz[:, hs], 2, C), op=Alu.mult)
        # ---- s-step: premul + PE block-diag ones reduction ----
        for ch in range(nchunk):
            sl = slice(ch * chs, (ch + 1) * chs)
            u4 = uq[ch]
            t1 = tmp.tile([128, chs, O, C], bf, tag="t2", bufs=3)
            nc.vector.tensor_tensor(
                out=t1[:], in0=u4[:],
                in1=bcast_ap(cw[:, sl, :], 2, O), op=Alu.mult)
            for k in range(chs // smm):
                s_idx = ch * chs + k * smm
                nc.tensor.matmul(
                    out=s_ps[:], lhsT=ones_sb[:, j, :],
                    rhs=t1[:, k * smm:(k + 1) * smm, :, :],
                    start=(j == 0 and s_idx == 0),
                    stop=(j == jlast and s_idx == nslab - smm))

    def collect_s(s_ps, nrow):
        """Sum the smm slab-positions -> s_pair [nrow, CO] ((o,c) order)."""
        s_pair = small.tile([nrow, CO], f32, tag="s_pair", bufs=2)
        if smm == 2:
            s_rw = small.tile([nrow, 2 * CO], f32, tag="s_rw")
            nc.scalar.copy(s_rw[:], s_ps[0:nrow, :])
            nc.vector.tensor_tensor(
                out=s_pair[:], in0=s_rw[:, 0:CO], in1=s_rw[:, CO:2 * CO],
                op=Alu.add)
        else:
            nc.scalar.copy(s_pair[:], s_ps[0:nrow, 0:CO])
        return s_pair

    def emit_final(s_ps, nrow, row0):
        """Deferred final squash + output DMA for a pair."""
        s_pair = collect_s(s_ps, nrow)
        v_sb = squash(s_pair, nrow, ROUTING_ITERS - 1)
        nc.sync.dma_start(
            out=vout_dram[row0:row0 + nrow, :], in_=v_sb[:])

    # Group 0's first quarter is emitted BEFORE the s1 sweep so its PE/DMA
    # work overlaps the sweep and routing can start right after s1's squash.
    ua0 = phase_a(0, 0, 1)
    v_bf1 = emit_s1()

    # Pair loop; the previous pair's FINAL squash is emitted after this
    # pair's it1 routing so it fills the squash/vrep stall.
    pending_final = None
    for j in range(npair):
        ga, gb = 2 * j, 2 * j + 1
        two = gb < ngroup
        nrow = 2 * BT if two else BT
        jlast = 1 if two else 0
        ua = (ua0 + phase_a(ga, 1)) if j == 0 else phase_a(ga)
        ub = phase_a(gb) if two else None
        Va = state.tile([128, O, C], bf, tag="vrep", name="vrep", bufs=4)
        load_vrep(Va, v_bf1, ga * BT)
        if two:
            Vb = state.tile([128, O, C], bf, tag="vrep", name="vrepb",
                            bufs=4)
            load_vrep(Vb, v_bf1, gb * BT)
        for it in range(1, ROUTING_ITERS):
            s_ps = psum_acc.tile([2 * BT, smm * CO], f32, tag="sps",
                                 name="sps")
            route_core(ga, ua, Va, s_ps, 0, jlast)
            if two:
                route_core(gb, ub, Vb, s_ps, 1, jlast)
            if pending_final is not None:
                emit_final(*pending_final)
                pending_final = None
            last = (it == ROUTING_ITERS - 1)
            if last:
                pending_final = (s_ps, nrow, ga * BT)
            else:
                s_pair = collect_s(s_ps, nrow)
                v_bf = squash(s_pair, nrow, it, vk=v1k[j])
                Va = state.tile([128, O, C], bf, tag="vrep", name="vrep2",
                                bufs=4)
                load_vrep(Va, v_bf, 0)
                if two:
                    Vb = state.tile([128, O, C], bf, tag="vrep",
                                    name="vrep2b", bufs=4)
                    load_vrep(Vb, v_bf, BT)
    if pending_final is not None:
        emit_final(*pending_final)


def make_inputs_per_core(u, W):
    """Full inputs -> list of 8 in_maps."""
    W0 = np.asarray(W, dtype=np.float32)[0]
    u = np.asarray(u, dtype=np.float32)
    in_maps = []
    for c in range(NCORES):
        u_core = u[c * B_LOC:(c + 1) * B_LOC]
        in_maps.append(_host_prep(u_core, W0))
    return in_maps


def numpy_model(u_core, W0):
    """f32 numpy model of the routing (for small-scale checks)."""
    u_hat = np.einsum('pcoi,bpi->bpco', W0, u_core)
    b = np.zeros(u_hat.shape[:3], dtype=np.float32)
    v = None
    for _ in range(ROUTING_ITERS):
        e = np.exp(b - b.max(axis=2, keepdims=True))
        c = e / e.sum(axis=2, keepdims=True)
        s = np.einsum('bpc,bpco->bco', c, u_hat)
        sq = (s * s).sum(-1, keepdims=True)
        v = (sq / (1 + sq)) * s / np.sqrt(sq + EPS)
        b = b + np.einsum('bpco,bco->bpc', u_hat, v)
    return v


_COMPILED = {}


def _get_compiled():
    if "nc" in _COMPILED:
        return _COMPILED["nc"]
    from contextlib import ExitStack
    import concourse.tile as tile
    from concourse import bacc

    nc = bacc.Bacc("TRN2", target_bir_lowering=False, debug=False,
                   num_devices=NCORES)
    with tile.TileContext(nc) as tc:
        with ExitStack() as ctx:
            build(nc, tc, ctx)
    nc.compile()
    _COMPILED["nc"] = nc
    return nc


def kernel(u, W):
    """Full-input entry point: u [256,2048,8] f32, W [1,2048,10,16,8] f32
    -> v [256, 10, 16] f32."""
    from concourse.bass_utils import run_bass_kernel_spmd

    nc = _get_compiled()
    in_maps = make_inputs_per_core(u, W)
    res = run_bass_kernel_spmd(nc, in_maps, core_ids=list(range(NCORES)))
    outs = [res.results[c]["v_out"] for c in range(NCORES)]
    # v_out rows are (o,c)-major -> [B, O, C] -> transpose to [B, C, O]
    v = np.concatenate(outs, axis=0).reshape(B, O, C)
    return np.ascontiguousarray(v.transpose(0, 2, 1)).astype(np.float32)


# revision 33
# speedup vs baseline: 1.2032x; 1.1267x over previous
"""Trainium2 Bass kernel for DigitCapsules dynamic routing (v4).

Problem: u [256, 2048, 8] f32, W [1, 2048, 10, 16, 8] f32
  u_hat = einsum('pcoi,bpi->bpco', W[0], u)
  3 routing iterations (softmax over c, weighted sum over p, squash,
  agreement update) -> v [256, 10, 16] f32.

Strategy (8 cores, data-parallel over batch, 32 batch elems per core):
  - Partition layout: slabs of 16 p-values; SBUF partition index =
    (p_local * 8 + b_member); the PE contraction runs over
    K = (p_local 16, i 8) = 128 via a block-diagonal stationary u_bd
    (host-built; the zeros cost nothing at matmul time).
  - W columns are (o,c)-ordered host-side, so PSUM arrives as
    [slab, o, c] and evacuation is a straight (non-transposing) copy.
  - u_hat is materialized as FOUR quarter tiles per group ([128, 32, O,
    C] bf16, c innermost) so routing starts as soon as the first
    quarter lands instead of waiting for all 128 slabs.
  - Iteration 1 needs no u_hat read: s1 = 0.1 * sum_p u_hat from a
    dense-u stationary matmul accumulated over all slabs.
  - Iterations 2,3: G via DVE mul + add-tree over o; softmax over c via
    ACT exp + DVE reduce; weighted s-sum via PE matmuls with TWO
    block-diagonal ones stationaries that accumulate the group pair
    into one [16, 320] PSUM tile (squash is pair-batched).
  - Squash: one-Newton rsqrt (bit-trick seed), ~15 DVE ops per pair.
  - v never roundtrips through DRAM: V_rep replication and the v1
    accumulator are SBUF->SBUF block-copy DMAs.
  - The first pair's FINAL squash is emitted after the second pair's
    it1 routing so it fills the second pair's squash/vrep stall.
"""

import numpy as np
import ml_dtypes

bf16 = ml_dtypes.bfloat16

# Problem constants (fixed by the problem spec; do not read spec.json here)
B, P, C, O, IN = 256, 2048, 10, 16, 8
NCORES = 8
B_LOC = B // NCORES          # 32 batch elems per core
BT = 8                       # batch elems per group (one octet)
NGROUP = B_LOC // BT         # 4 groups per core
PSLAB = 16                   # p-values per slab
NSLAB = P // PSLAB           # 128 slabs
CO = C * O                   # 160
ROUTING_ITERS = 3
EPS = 1e-9

CHB = 8     # slabs per u_bd DMA chunk
EV = 3      # slabs per PSUM evacuation batch (1 bank per tile)
SMM = 2     # slabs per s-step matmul (N = SMM*CO = 320 <= 512)
CHS = 32    # slabs per routing compute chunk (= uhat quarter size)


def _host_prep(u_core, W0, nslab=NSLAB, ngroup=NGROUP):
    """Build host-side reordered (k-major, contiguous-DMA) arrays."""
    # w_k[p*8+i, s, o*10+c] = W0[16s+p, c, o, i]  ((o,c)-ordered columns)
    w = W0.reshape(nslab, PSLAB, C, O, IN)
    w_k = np.ascontiguousarray(
        w.transpose(1, 4, 0, 3, 2).reshape(PSLAB * IN, nslab, CO)
    ).astype(bf16)

    # x[g, b, s, p, i] = u_core[g*8 + b, 16s+p, i]
    x = u_core.reshape(ngroup, BT, nslab, PSLAB, IN)

    # ubd_k[g, ci, p*8+i, s_in_chunk, p'*8+b] = x[g,b,ci*chb+s,p,i]*(p==p')
    # -- chunk-major so every u_bd DMA is one fully contiguous read
    xt = x.transpose(0, 3, 4, 2, 1)  # [g, p, i, s, b]
    ubd_k = np.zeros((ngroup, PSLAB, IN, nslab, PSLAB, BT), dtype=bf16)
    for p in range(PSLAB):
        ubd_k[:, p, :, :, p, :] = xt[:, p]
    chb = min(CHB, nslab)
    ubd_k = np.ascontiguousarray(
        ubd_k.reshape(ngroup, PSLAB * IN, nslab // chb, chb, PSLAB * BT)
        .transpose(0, 2, 1, 3, 4))

    # ut_k[p*8+i, s, g*8+b] = x[g,b,s,p,i] -- one dense stationary for the
    # s1 matmul covering ALL groups (M = ngroup*BT)
    ut_k = np.ascontiguousarray(
        x.transpose(3, 4, 2, 0, 1).reshape(PSLAB * IN, nslab, ngroup * BT)
    ).astype(bf16)

    # ones2[j, p*8+b, j'*8+b'] = (b == b') & (j == j') -- stationaries for
    # the s-reduction; group j of a pair fills output rows j*8..j*8+8 while
    # contributing zeros to the other half (PSUM-accumulated jointly).
    ones2 = np.zeros((2, PSLAB * BT, 2 * BT), dtype=bf16)
    for j in range(2):
        for p in range(PSLAB):
            for b in range(BT):
                ones2[j, p * BT + b, j * BT + b] = 1.0
    return {
        "w_k": w_k,
        "ubd_k": ubd_k,
        "ut_k": ut_k,
        "ones2": ones2,
    }


def build(nc, tc, ctx, nslab=NSLAB, ngroup=NGROUP):
    """Emit the kernel IR. Parameterized slab/group counts for small tests."""
    import concourse.bass as bass
    from concourse import mybir

    f32 = mybir.dt.float32
    i32 = mybir.dt.int32
    bf = mybir.dt.bfloat16
    Alu = mybir.AluOpType
    Act = mybir.ActivationFunctionType
    Ax = mybir.AxisListType

    b_loc = ngroup * BT
    chb = min(CHB, nslab)
    ev = min(EV, nslab)
    smm = min(SMM, nslab)
    chs = min(CHS, nslab)
    nq = (nslab + chs - 1) // chs   # quarters per group

    # ---- DRAM parameters ----
    w_dram = nc.dram_tensor(
        "w_k", [PSLAB * IN, nslab, CO], bf, kind="ExternalInput").ap()
    ubd_dram = nc.dram_tensor(
        "ubd_k", [ngroup, nslab // min(CHB, nslab), PSLAB * IN,
                  min(CHB, nslab), PSLAB * BT], bf,
        kind="ExternalInput").ap()
    ut_dram = nc.dram_tensor(
        "ut_k", [PSLAB * IN, nslab, ngroup * BT], bf,
        kind="ExternalInput").ap()
    ones_dram = nc.dram_tensor(
        "ones2", [2, PSLAB * BT, 2 * BT], bf, kind="ExternalInput").ap()
    vout_dram = nc.dram_tensor(
        "v_out", [b_loc, CO], f32, kind="ExternalOutput").ap()

    # ---- pools ----
    consts = ctx.enter_context(tc.tile_pool(name="consts", bufs=1))
    ubdpool = ctx.enter_context(tc.tile_pool(name="ubdpool", bufs=2))
    utpool = ctx.enter_context(tc.tile_pool(name="utpool", bufs=1))
    uhatpool = ctx.enter_context(tc.tile_pool(name="uhat", bufs=2 * nq))
    psum = ctx.enter_context(tc.tile_pool(name="psum", bufs=4, space="PSUM"))
    psum_acc = ctx.enter_context(
        tc.tile_pool(name="psum_acc", bufs=2, space="PSUM"))
    small = ctx.enter_context(tc.tile_pool(name="small", bufs=2))
    state = ctx.enter_context(tc.tile_pool(name="state", bufs=2))
    tmp = ctx.enter_context(tc.tile_pool(name="tmp", bufs=2))

    ones_sb = consts.tile([PSLAB * BT, 2, 2 * BT], bf)
    for j in range(2):
        nc.sync.dma_start(out=ones_sb[:, j, :], in_=ones_dram[j])
    magic = consts.tile([128, 1], i32)
    nc.gpsimd.memset(magic[:], 0x5F3759DF)

    # v1 accumulator per pair [16, O*C] (o,c)-major bf16
    npair = (ngroup + 1) // 2
    v1k = [consts.tile([2 * BT, O * C], bf, name=f"v1k{j}")
           for j in range(npair)]

    # resident W: whole tensor, contiguous quarters across the queues
    wall = consts.tile([PSLAB * IN, nslab, CO], bf)
    h = max(1, nslab // 4)
    engs = [nc.scalar, nc.sync, nc.gpsimd, nc.scalar]
    for jj, j in enumerate(range(0, nslab, h)):
        engs[jj % 4].dma_start(
            out=wall[:, j:j + h, :], in_=w_dram[:, j:j + h, :])

    def bcast_ap(ap, insert_pos, size):
        """Insert a stride-0 dim of `size` at free-dim position insert_pos."""
        new = list(ap.ap)
        new.insert(insert_pos, [0, size])
        return bass.AP(tensor=ap.tensor, offset=ap.offset, ap=new)

    def squash(s_sb, n, it, vk=None):
        """s_sb: [n, CO] f32, (o,c)-major. it < last: returns v_bf [n, O*C]
        bf16 (o,c) with accumulated V (vk + v) when vk given; else final:
        returns v_sb f32 (o,c).  factor = nrm/((1+nrm)*sqrt(nrm+eps)),
        rsqrt via bit-trick seed + one Newton step (~0.2% rel err)."""
        s3 = s_sb[:].rearrange("n (o c) -> n c o", o=O)
        sq = small.tile([n, CO], f32, tag="sqsq", bufs=1)
        nc.vector.tensor_mul(sq[:].rearrange("n (o c) -> n c o", o=O), s3, s3)
        nrm = small.tile([n, C], f32, tag="nrm")
        nc.vector.tensor_reduce(
            out=nrm[:], in_=sq[:].rearrange("n (o c) -> n c o", o=O),
            axis=Ax.X, op=Alu.add)
        d1 = small.tile([n, C], f32, tag="d1")
        nc.vector.tensor_scalar_add(d1[:], nrm[:], 1.0)
        r1 = small.tile([n, C], f32, tag="r1")
        nc.vector.reciprocal(r1[:], d1[:])
        se = small.tile([n, C], f32, tag="se")
        nc.vector.tensor_scalar_add(se[:], nrm[:], EPS)
        sh = small.tile([n, C], i32, tag="sh")
        nc.vector.tensor_scalar(
            out=sh[:], in0=se[:].bitcast(i32), scalar1=1, scalar2=None,
            op0=Alu.logical_shift_right)
        y0 = small.tile([n, C], i32, tag="y0")
        nc.vector.tensor_tensor(
            out=y0[:], in0=bcast_ap(magic[0:n, :], 1, C), in1=sh[:],
            op=Alu.subtract)
        y = y0[:].bitcast(f32)
        aa = small.tile([n, C], f32, tag="na")
        nc.vector.tensor_tensor(out=aa[:], in0=y, in1=y, op=Alu.mult)
        nc.vector.tensor_tensor(out=aa[:], in0=aa[:], in1=se[:],
                                op=Alu.mult)
        nc.vector.tensor_scalar(
            out=aa[:], in0=aa[:], scalar1=-0.5, scalar2=1.5,
            op0=Alu.mult, op1=Alu.add)
        yn = small.tile([n, C], f32, tag="ny")
        nc.vector.tensor_tensor(out=yn[:], in0=y, in1=aa[:], op=Alu.mult)
        f1 = small.tile([n, C], f32, tag="f1")
        nc.vector.tensor_mul(f1[:], nrm[:], r1[:])
        fac = small.tile([n, C], f32, tag="fac")
        nc.vector.tensor_mul(fac[:], f1[:], yn[:])
        v_sb = small.tile([n, CO], f32, tag="v_sb")
        nc.vector.tensor_tensor(
            out=v_sb[:].rearrange("n (o c) -> n c o", o=O),
            in0=s3, in1=bcast_ap(fac[:], 2, O), op=Alu.mult)
        if it == ROUTING_ITERS - 1:
            return v_sb
        v_bf = small.tile([n, O * C], bf, tag="v_bf")
        if vk is not None:
            # accumulated V = v1 + v2 so the next iteration's logits come
            # out of one linear G pass
            nc.vector.tensor_tensor(
                out=v_bf[:], in0=v_sb[:], in1=vk[0:n, :], op=Alu.add)
        else:
            nc.vector.tensor_copy(out=v_bf[:], in_=v_sb[:])
        return v_bf

    def load_vrep(V_rep, v_bf, r0):
        """Replicate v rows [r0:r0+8] across the 16 p-positions (16 small
        SBUF->SBUF block-copy DMAs spread over the queues)."""
        for p in range(PSLAB):
            eng = (nc.sync, nc.gpsimd, nc.scalar)[p % 3]
            eng.dma_start(
                out=V_rep[p * BT:(p + 1) * BT, :, :]
                    .rearrange("n o c -> n (o c)"),
                in_=v_bf[r0:r0 + BT, :])

    def emit_s1():
        # ------- s1 sweep: one accumulation for ALL groups (M=32) -------
        ut_res = utpool.tile([PSLAB * IN, nslab, ngroup * BT], bf,
                             tag="utres", name="ut_res")
        nc.sync.dma_start(out=ut_res[:], in_=ut_dram)
        s1_ps = psum_acc.tile([ngroup * BT, CO], f32, tag="s1ps",
                              name="s1ps")
        for s in range(nslab):
            nc.tensor.matmul(
                out=s1_ps[:], lhsT=ut_res[:, s, :], rhs=wall[:, s, :],
                start=(s == 0), stop=(s == nslab - 1))
        s1_sb = consts.tile([ngroup * BT, CO], f32)
        nc.scalar.mul(s1_sb[:], s1_ps[:], 1.0 / C)
        v_bf1 = squash(s1_sb, ngroup * BT, 0)
        for j in range(npair):
            r = min(2 * BT, b_loc - j * 2 * BT)
            nc.scalar.dma_start(out=v1k[j][0:r, :],
                                in_=v_bf1[j * 2 * BT:j * 2 * BT + r, :])
        return v_bf1

    def phase_a(g, q_from=0, q_to=None):
        """u_hat materialization for quarters [q_from, q_to); returns the
        quarter tiles."""
        if q_to is None:
            q_to = nq
        quarters = []
        ubs = {}

        def get_ub(ci):
            if ci not in ubs:
                ub = ubdpool.tile([PSLAB * IN, chb, PSLAB * BT], bf,
                                  tag="ubd", name="ubd")
                eng = nc.gpsimd if ci % 2 else nc.sync
                eng.dma_start(out=ub[:], in_=ubd_dram[g, ci])
                ubs[ci] = ub
            return ubs[ci]

        for q0 in range(q_from * chs, min(q_to * chs, nslab), chs):
            qn = min(chs, nslab - q0)
            uq = uhatpool.tile([128, chs, O, C], bf, tag="uhat",
                               name=f"uhat{g}_{q0}")
            quarters.append(uq)
            s0 = q0
            while s0 < q0 + qn:
                nb = min(ev, q0 + qn - s0)
                ps = psum.tile([128, ev, CO], f32, tag="ups", name="ups")
                get_ub(s0 // chb)
                get_ub((s0 + nb - 1) // chb)
                for qq in range(nb):
                    sl = s0 + qq
                    ub = ubs[sl // chb]
                    nc.tensor.matmul(
                        out=ps[:, qq, :], lhsT=ub[:, sl % chb, :],
                        rhs=wall[:, sl, :], start=True, stop=True)
                # (o,c)-ordered W columns -> straight PSUM->SBUF copy
                nc.scalar.copy(
                    uq[:, s0 - q0:s0 - q0 + nb, :, :]
                        .rearrange("p s o c -> p (s o c)"),
                    ps[:, 0:nb, :].rearrange("p s x -> p (s x)"))
                s0 += nb
        return quarters

    def route_core(g, uq, V_rep, s_ps, j, jlast):
        """One group's G/softmax/premul/s-matmul for one iteration.
        uq: list of uhat quarter tiles. The s-matmuls accumulate into the
        pair-shared s_ps [16, smm*CO] via the ones2[j] stationary."""
        # b-state scratch (bf16: G magnitudes are <<1)
        bst = state.tile([128, nslab, C], bf, tag="bst", name="bst")
        nchunk = nq

        # ---- G-step: bst = sum_o uhat * V_rep ----
        for ch in range(nchunk):
            sl = slice(ch * chs, (ch + 1) * chs)
            u4 = uq[ch]
            t2 = tmp.tile([128, chs, O, C], bf, tag="t2", bufs=3)
            nc.vector.tensor_tensor(
                out=t2[:], in0=u4[:],
                in1=bcast_ap(V_rep[:], 1, chs), op=Alu.mult)
            r1 = tmp.tile([128, chs, O // 2, C], bf, tag="r1t")
            nc.vector.tensor_tensor(
                out=r1[:], in0=t2[:, :, 0:O // 2, :],
                in1=t2[:, :, O // 2:O, :], op=Alu.add)
            r2 = tmp.tile([128, chs, O // 4, C], bf, tag="r2t")
            nc.vector.tensor_tensor(
                out=r2[:], in0=r1[:, :, 0:O // 4, :],
                in1=r1[:, :, O // 4:O // 2, :], op=Alu.add)
            r3 = tmp.tile([128, chs, 2, C], bf, tag="r3t")
            nc.vector.tensor_tensor(
                out=r3[:], in0=r2[:, :, 0:2, :],
                in1=r2[:, :, 2:4, :], op=Alu.add)
            nc.vector.tensor_tensor(
                out=bst[:, sl, :], in0=r3[:, :, 0, :],
                in1=r3[:, :, 1, :], op=Alu.add)
        # ---- softmax over c (2 coarse chunks to cut instr count) ----
        expt = tmp.tile([128, nslab, C], bf, tag="expt", bufs=2)
        Z = tmp.tile([128, nslab], f32, tag="Z")
        rz = tmp.tile([128, nslab], f32, tag="rz")
        cw = tmp.tile([128, nslab, C], bf, tag="cw", bufs=2)
        nsoft = min(2, nchunk)
        sft = nslab // nsoft
        for hf in range(nsoft):
            hs = slice(hf * sft, (hf + 1) * sft)
            nc.scalar.activation(expt[:, hs, :], bst[:, hs, :], Act.Exp)
            nc.vector.tensor_reduce(
                out=Z[:, hs], in_=expt[:, hs, :], axis=Ax.X, op=Alu.add)
            nc.vector.reciprocal(rz[:, hs], Z[:, hs])
            nc.vector.tensor_tensor(
                out=cw[:, hs, :], in0=expt[:, hs, :],
                in1=bcast_ap(rz[:, hs], 2, C), op=Alu.mult)
        # ---- s-step: premul + PE block-diag ones reduction ----
        for ch in range(nchunk):
            sl = slice(ch * chs, (ch + 1) * chs)
            u4 = uq[ch]
            t1 = tmp.tile([128, chs, O, C], bf, tag="t2", bufs=3)
            nc.vector.tensor_tensor(
                out=t1[:], in0=u4[:],
                in1=bcast_ap(cw[:, sl, :], 2, O), op=Alu.mult)
            for k in range(chs // smm):
                s_idx = ch * chs + k * smm
                nc.tensor.matmul(
                    out=s_ps[:], lhsT=ones_sb[:, j, :],
                    rhs=t1[:, k * smm:(k + 1) * smm, :, :],
                    start=(j == 0 and s_idx == 0),
                    stop=(j == jlast and s_idx == nslab - smm))

    def collect_s(s_ps, nrow):
        """Sum the smm slab-positions -> s_pair [nrow, CO] ((o,c) order)."""
        s_pair = small.tile([nrow, CO], f32, tag="s_pair", bufs=2)
        if smm == 2:
            s_rw = small.tile([nrow, 2 * CO], f32, tag="s_rw")
            nc.scalar.copy(s_rw[:], s_ps[0:nrow, :])
            nc.vector.tensor_tensor(
                out=s_pair[:], in0=s_rw[:, 0:CO], in1=s_rw[:, CO:2 * CO],
                op=Alu.add)
        else:
            nc.scalar.copy(s_pair[:], s_ps[0:nrow, 0:CO])
        return s_pair

    def emit_final(s_ps, nrow, row0):
        """Deferred final squash + output DMA for a pair."""
        s_pair = collect_s(s_ps, nrow)
        v_sb = squash(s_pair, nrow, ROUTING_ITERS - 1)
        nc.sync.dma_start(
            out=vout_dram[row0:row0 + nrow, :], in_=v_sb[:])

    v_bf1 = emit_s1()

    # Pair loop; the previous pair's FINAL squash is emitted after this
    # pair's it1 routing so it fills the squash/vrep stall.
    pending_final = None
    for j in range(npair):
        ga, gb = 2 * j, 2 * j + 1
        two = gb < ngroup
        nrow = 2 * BT if two else BT
        jlast = 1 if two else 0
        ua = phase_a(ga)
        ub = phase_a(gb) if two else None
        Va = state.tile([128, O, C], bf, tag="vrep", name="vrep", bufs=4)
        load_vrep(Va, v_bf1, ga * BT)
        if two:
            Vb = state.tile([128, O, C], bf, tag="vrep", name="vrepb",
                            bufs=4)
            load_vrep(Vb, v_bf1, gb * BT)
        for it in range(1, ROUTING_ITERS):
            s_ps = psum_acc.tile([2 * BT, smm * CO], f32, tag="sps",
                                 name="sps")
            route_core(ga, ua, Va, s_ps, 0, jlast)
            if two:
                route_core(gb, ub, Vb, s_ps, 1, jlast)
            if pending_final is not None:
                emit_final(*pending_final)
                pending_final = None
            last = (it == ROUTING_ITERS - 1)
            if last:
                pending_final = (s_ps, nrow, ga * BT)
            else:
                s_pair = collect_s(s_ps, nrow)
                v_bf = squash(s_pair, nrow, it, vk=v1k[j])
                Va = state.tile([128, O, C], bf, tag="vrep", name="vrep2",
                                bufs=4)
                load_vrep(Va, v_bf, 0)
                if two:
                    Vb = state.tile([128, O, C], bf, tag="vrep",
                                    name="vrep2b", bufs=4)
                    load_vrep(Vb, v_bf, BT)
    if pending_final is not None:
        emit_final(*pending_final)


def make_inputs_per_core(u, W):
    """Full inputs -> list of 8 in_maps."""
    W0 = np.asarray(W, dtype=np.float32)[0]
    u = np.asarray(u, dtype=np.float32)
    in_maps = []
    for c in range(NCORES):
        u_core = u[c * B_LOC:(c + 1) * B_LOC]
        in_maps.append(_host_prep(u_core, W0))
    return in_maps


def numpy_model(u_core, W0):
    """f32 numpy model of the routing (for small-scale checks)."""
    u_hat = np.einsum('pcoi,bpi->bpco', W0, u_core)
    b = np.zeros(u_hat.shape[:3], dtype=np.float32)
    v = None
    for _ in range(ROUTING_ITERS):
        e = np.exp(b - b.max(axis=2, keepdims=True))
        c = e / e.sum(axis=2, keepdims=True)
        s = np.einsum('bpc,bpco->bco', c, u_hat)
        sq = (s * s).sum(-1, keepdims=True)
        v = (sq / (1 + sq)) * s / np.sqrt(sq + EPS)
        b = b + np.einsum('bpco,bco->bpc', u_hat, v)
    return v


_COMPILED = {}


def _get_compiled():
    if "nc" in _COMPILED:
        return _COMPILED["nc"]
    from contextlib import ExitStack
    import concourse.tile as tile
    from concourse import bacc

    nc = bacc.Bacc("TRN2", target_bir_lowering=False, debug=False,
                   num_devices=NCORES)
    with tile.TileContext(nc) as tc:
        with ExitStack() as ctx:
            build(nc, tc, ctx)
    nc.compile()
    _COMPILED["nc"] = nc
    return nc


def kernel(u, W):
    """Full-input entry point: u [256,2048,8] f32, W [1,2048,10,16,8] f32
    -> v [256, 10, 16] f32."""
    from concourse.bass_utils import run_bass_kernel_spmd

    nc = _get_compiled()
    in_maps = make_inputs_per_core(u, W)
    res = run_bass_kernel_spmd(nc, in_maps, core_ids=list(range(NCORES)))
    outs = [res.results[c]["v_out"] for c in range(NCORES)]
    # v_out rows are (o,c)-major -> [B, O, C] -> transpose to [B, C, O]
    v = np.concatenate(outs, axis=0).reshape(B, O, C)
    return np.ascontiguousarray(v.transpose(0, 2, 1)).astype(np.float32)


# revision 52
# speedup vs baseline: 1.3197x; 1.0968x over previous
"""Trainium2 Bass kernel for DigitCapsules dynamic routing (v4).

Problem: u [256, 2048, 8] f32, W [1, 2048, 10, 16, 8] f32
  u_hat = einsum('pcoi,bpi->bpco', W[0], u)
  3 routing iterations (softmax over c, weighted sum over p, squash,
  agreement update) -> v [256, 10, 16] f32.

Strategy (8 cores, data-parallel over batch, 32 batch elems per core):
  - Partition layout: slabs of 16 p-values; SBUF partition index =
    (p_local * 8 + b_member); the PE contraction runs over
    K = (p_local 16, i 8) = 128 via a block-diagonal stationary u_bd
    (host-built; the zeros cost nothing at matmul time).
  - W columns are (o,c)-ordered host-side, so PSUM arrives as
    [slab, o, c] and evacuation is a straight (non-transposing) copy.
  - u_hat is materialized as FOUR quarter tiles per group ([128, 32, O,
    C] bf16, c innermost) so routing starts as soon as the first
    quarter lands instead of waiting for all 128 slabs.
  - Iteration 1 needs no u_hat read: s1 = 0.1 * sum_p u_hat from a
    dense-u stationary matmul accumulated over all slabs.
  - Iterations 2,3: G via DVE mul + add-tree over o; softmax over c via
    ACT exp + DVE reduce; weighted s-sum via PE matmuls with TWO
    block-diagonal ones stationaries that accumulate the group pair
    into one [16, 320] PSUM tile (squash is pair-batched).
  - Squash: one-Newton rsqrt (bit-trick seed), ~15 DVE ops per pair.
  - v never roundtrips through DRAM: V_rep replication and the v1
    accumulator are SBUF->SBUF block-copy DMAs.
  - The first pair's FINAL squash is emitted after the second pair's
    it1 routing so it fills the second pair's squash/vrep stall.
"""

import numpy as np
import ml_dtypes

bf16 = ml_dtypes.bfloat16

# Problem constants (fixed by the problem spec; do not read spec.json here)
B, P, C, O, IN = 256, 2048, 10, 16, 8
NCORES = 8
B_LOC = B // NCORES          # 32 batch elems per core
BT = 8                       # batch elems per group (one octet)
NGROUP = B_LOC // BT         # 4 groups per core
PSLAB = 16                   # p-values per slab
NSLAB = P // PSLAB           # 128 slabs
CO = C * O                   # 160
ROUTING_ITERS = 3
EPS = 1e-9

CHB = 8     # slabs per u_bd DMA chunk
EV = 3      # slabs per PSUM evacuation batch (1 bank per tile)
SMM = 2     # slabs per s-step matmul (N = SMM*CO = 320 <= 512)
CHS = 32    # slabs per routing compute chunk (= uhat quarter size)


def _host_prep(u_core, W0, nslab=NSLAB, ngroup=NGROUP):
    """Build host-side reordered (k-major, contiguous-DMA) arrays."""
    # w_k[p*8+i, s, o*10+c] = W0[16s+p, c, o, i]  ((o,c)-ordered columns)
    w = W0.reshape(nslab, PSLAB, C, O, IN)
    w_k = np.ascontiguousarray(
        w.transpose(1, 4, 0, 3, 2).reshape(PSLAB * IN, nslab, CO)
    ).astype(bf16)

    # x[g, b, s, p, i] = u_core[g*8 + b, 16s+p, i]
    x = u_core.reshape(ngroup, BT, nslab, PSLAB, IN)

    # ubd_k[g, ci, p*8+i, s_in_chunk, p'*8+b] = x[g,b,ci*chb+s,p,i]*(p==p')
    # -- chunk-major so every u_bd DMA is one fully contiguous read
    xt = x.transpose(0, 3, 4, 2, 1)  # [g, p, i, s, b]
    ubd_k = np.zeros((ngroup, PSLAB, IN, nslab, PSLAB, BT), dtype=bf16)
    for p in range(PSLAB):
        ubd_k[:, p, :, :, p, :] = xt[:, p]
    chb = min(CHB, nslab)
    ubd_k = np.ascontiguousarray(
        ubd_k.reshape(ngroup, PSLAB * IN, nslab // chb, chb, PSLAB * BT)
        .transpose(0, 2, 1, 3, 4))

    # ut_k[p*8+i, s, g*8+b] = x[g,b,s,p,i] -- one dense stationary for the
    # s1 matmul covering ALL groups (M = ngroup*BT)
    ut_k = np.ascontiguousarray(
        x.transpose(3, 4, 2, 0, 1).reshape(PSLAB * IN, nslab, ngroup * BT)
    ).astype(bf16)

    # ones2[j, p*8+b, j'*8+b'] = (b == b') & (j == j') -- stationaries for
    # the s-reduction; group j of a pair fills output rows j*8..j*8+8 while
    # contributing zeros to the other half (PSUM-accumulated jointly).
    ones2 = np.zeros((2, PSLAB * BT, 2 * BT), dtype=bf16)
    for j in range(2):
        for p in range(PSLAB):
            for b in range(BT):
                ones2[j, p * BT + b, j * BT + b] = 1.0
    return {
        "w_k": w_k,
        "ubd_k": ubd_k,
        "ut_k": ut_k,
        "ones2": ones2,
    }


def build(nc, tc, ctx, nslab=NSLAB, ngroup=NGROUP):
    """Emit the kernel IR. Parameterized slab/group counts for small tests."""
    import concourse.bass as bass
    from concourse import mybir

    f32 = mybir.dt.float32
    i32 = mybir.dt.int32
    bf = mybir.dt.bfloat16
    Alu = mybir.AluOpType
    Act = mybir.ActivationFunctionType
    Ax = mybir.AxisListType

    b_loc = ngroup * BT
    chb = min(CHB, nslab)
    ev = min(EV, nslab)
    smm = min(SMM, nslab)
    chs = min(CHS, nslab)
    nq = (nslab + chs - 1) // chs   # quarters per group

    # ---- DRAM parameters ----
    w_dram = nc.dram_tensor(
        "w_k", [PSLAB * IN, nslab, CO], bf, kind="ExternalInput").ap()
    ubd_dram = nc.dram_tensor(
        "ubd_k", [ngroup, nslab // min(CHB, nslab), PSLAB * IN,
                  min(CHB, nslab), PSLAB * BT], bf,
        kind="ExternalInput").ap()
    ut_dram = nc.dram_tensor(
        "ut_k", [PSLAB * IN, nslab, ngroup * BT], bf,
        kind="ExternalInput").ap()
    ones_dram = nc.dram_tensor(
        "ones2", [2, PSLAB * BT, 2 * BT], bf, kind="ExternalInput").ap()
    vout_dram = nc.dram_tensor(
        "v_out", [b_loc, CO], f32, kind="ExternalOutput").ap()

    # ---- pools ----
    consts = ctx.enter_context(tc.tile_pool(name="consts", bufs=1))
    ubdpool = ctx.enter_context(tc.tile_pool(name="ubdpool", bufs=2))
    utpool = ctx.enter_context(tc.tile_pool(name="utpool", bufs=1))
    uhatpool = ctx.enter_context(tc.tile_pool(name="uhat", bufs=2 * nq))
    psum = ctx.enter_context(tc.tile_pool(name="psum", bufs=4, space="PSUM"))
    psum_acc = ctx.enter_context(
        tc.tile_pool(name="psum_acc", bufs=2, space="PSUM"))
    small = ctx.enter_context(tc.tile_pool(name="small", bufs=2))
    state = ctx.enter_context(tc.tile_pool(name="state", bufs=2))
    tmp = ctx.enter_context(tc.tile_pool(name="tmp", bufs=2))

    ones_sb = consts.tile([PSLAB * BT, 2, 2 * BT], bf)
    for j in range(2):
        nc.sync.dma_start(out=ones_sb[:, j, :], in_=ones_dram[j])
    magic = consts.tile([128, 1], i32)
    nc.gpsimd.memset(magic[:], 0x5F3759DF)


    # v1 accumulator per pair [16, O*C] (o,c)-major bf16
    npair = (ngroup + 1) // 2
    v1k = [consts.tile([2 * BT, O * C], bf, name=f"v1k{j}")
           for j in range(npair)]

    # resident W: whole tensor, ~256KB pieces round-robin over the queues
    # (each dma_start lands on one HW DMA engine at ~30GB/s; small pieces
    # spread the load so the first slabs arrive fast)
    wall = consts.tile([PSLAB * IN, nslab, CO], bf)

    def bcast_ap(ap, insert_pos, size):
        """Insert a stride-0 dim of `size` at free-dim position insert_pos."""
        new = list(ap.ap)
        new.insert(insert_pos, [0, size])
        return bass.AP(tensor=ap.tensor, offset=ap.offset, ap=new)

    def squash(s_sb, n, it, vk=None):
        """s_sb: [n, CO] f32, (o,c)-major. it < last: returns v_bf [n, O*C]
        bf16 (o,c) with accumulated V (vk + v) when vk given; else final:
        returns v_sb f32 (o,c).  factor = nrm/((1+nrm)*sqrt(nrm+eps)),
        rsqrt via bit-trick seed + one Newton step (~0.2% rel err)."""
        s3 = s_sb[:].rearrange("n (o c) -> n c o", o=O)
        sq = small.tile([n, CO], f32, tag="sqsq", bufs=1)
        nc.vector.tensor_mul(sq[:].rearrange("n (o c) -> n c o", o=O), s3, s3)
        nrm = small.tile([n, C], f32, tag="nrm")
        nc.vector.tensor_reduce(
            out=nrm[:], in_=sq[:].rearrange("n (o c) -> n c o", o=O),
            axis=Ax.X, op=Alu.add)
        d1 = small.tile([n, C], f32, tag="d1")
        nc.vector.tensor_scalar_add(d1[:], nrm[:], 1.0)
        r1 = small.tile([n, C], f32, tag="r1")
        nc.vector.reciprocal(r1[:], d1[:])
        se = small.tile([n, C], f32, tag="se")
        nc.vector.tensor_scalar_add(se[:], nrm[:], EPS)
        sh = small.tile([n, C], i32, tag="sh")
        nc.vector.tensor_scalar(
            out=sh[:], in0=se[:].bitcast(i32), scalar1=1, scalar2=None,
            op0=Alu.logical_shift_right)
        y0 = small.tile([n, C], i32, tag="y0")
        nc.vector.tensor_tensor(
            out=y0[:], in0=bcast_ap(magic[0:n, :], 1, C), in1=sh[:],
            op=Alu.subtract)
        y = y0[:].bitcast(f32)
        aa = small.tile([n, C], f32, tag="na")
        nc.vector.tensor_tensor(out=aa[:], in0=y, in1=y, op=Alu.mult)
        nc.vector.tensor_tensor(out=aa[:], in0=aa[:], in1=se[:],
                                op=Alu.mult)
        nc.vector.tensor_scalar(
            out=aa[:], in0=aa[:], scalar1=-0.5, scalar2=1.5,
            op0=Alu.mult, op1=Alu.add)
        yn = small.tile([n, C], f32, tag="ny")
        nc.vector.tensor_tensor(out=yn[:], in0=y, in1=aa[:], op=Alu.mult)
        f1 = small.tile([n, C], f32, tag="f1")
        nc.vector.tensor_mul(f1[:], nrm[:], r1[:])
        fac = small.tile([n, C], f32, tag="fac")
        nc.vector.tensor_mul(fac[:], f1[:], yn[:])
        v_sb = small.tile([n, CO], f32, tag="v_sb")
        nc.vector.tensor_tensor(
            out=v_sb[:].rearrange("n (o c) -> n c o", o=O),
            in0=s3, in1=bcast_ap(fac[:], 2, O), op=Alu.mult)
        if it == ROUTING_ITERS - 1:
            return v_sb
        v_bf = small.tile([n, O * C], bf, tag="v_bf")
        if vk is not None:
            # accumulated V = v1 + v2 so the next iteration's logits come
            # out of one linear G pass
            nc.vector.tensor_tensor(
                out=v_bf[:], in0=v_sb[:], in1=vk[0:n, :], op=Alu.add)
        else:
            nc.vector.tensor_copy(out=v_bf[:], in_=v_sb[:])
        return v_bf

    def load_vrep(V_rep, v_bf, r0):
        """Replicate v rows [r0:r0+8] across the 16 p-positions (16 small
        SBUF->SBUF block-copy DMAs spread over the queues)."""
        for p in range(PSLAB):
            eng = (nc.sync, nc.gpsimd)[p % 2]
            eng.dma_start(
                out=V_rep[p * BT:(p + 1) * BT, :, :]
                    .rearrange("n o c -> n (o c)"),
                in_=v_bf[r0:r0 + BT, :])

    # ut first (s1 stationary), then W, round-robin over the rings
    ut_res = utpool.tile([PSLAB * IN, nslab, ngroup * BT], bf,
                         tag="utres", name="ut_res")
    engs3 = [nc.scalar, nc.sync, nc.gpsimd]
    hu = max(1, nslab // 4)
    for jj, j in enumerate(range(0, nslab, hu)):
        engs3[jj % 3].dma_start(
            out=ut_res[:, j:j + hu, :], in_=ut_dram[:, j:j + hu, :])
    h = max(1, nslab // 16)
    for jj, j in enumerate(range(0, nslab, h)):
        engs3[jj % 3].dma_start(
            out=wall[:, j:j + h, :], in_=w_dram[:, j:j + h, :])

    def emit_s1():
        # ------- s1 sweep: one accumulation for ALL groups (M=32) -------
        s1_ps = psum_acc.tile([ngroup * BT, CO], f32, tag="s1ps",
                              name="s1ps", bufs=1)
        for s in range(nslab):
            nc.tensor.matmul(
                out=s1_ps[:], lhsT=ut_res[:, s, :], rhs=wall[:, s, :],
                start=(s == 0), stop=(s == nslab - 1))
        s1_sb = consts.tile([ngroup * BT, CO], f32)
        nc.scalar.mul(s1_sb[:], s1_ps[:], 1.0 / C)
        v_bf1 = squash(s1_sb, ngroup * BT, 0)
        for j in range(npair):
            r = min(2 * BT, b_loc - j * 2 * BT)
            nc.sync.dma_start(out=v1k[j][0:r, :],
                                in_=v_bf1[j * 2 * BT:j * 2 * BT + r, :])
        return v_bf1

    def phase_a(g, q_from=0, q_to=None, evac_engs=None):
        """u_hat materialization for quarters [q_from, q_to); returns the
        quarter tiles. evac_engs: per-quarter engine for the PSUM->SBUF
        evacuation copy (defaults to scalar)."""
        if q_to is None:
            q_to = nq
        quarters = []
        ubs = {}

        def get_ub(ci):
            if ci not in ubs:
                ub = ubdpool.tile([PSLAB * IN, chb, PSLAB * BT], bf,
                                  tag="ubd", name="ubd")
                eng = nc.gpsimd if ci % 2 else nc.sync
                eng.dma_start(out=ub[:], in_=ubd_dram[g, ci])
                ubs[ci] = ub
            return ubs[ci]

        for qi, q0 in enumerate(
                range(q_from * chs, min(q_to * chs, nslab), chs)):
            qn = min(chs, nslab - q0)
            uq = uhatpool.tile([128, chs, O, C], bf, tag="uhat",
                               name=f"uhat{g}_{q0}")
            quarters.append(uq)
            ee = (evac_engs[qi] if evac_engs is not None
                  else nc.scalar)
            s0 = q0
            while s0 < q0 + qn:
                nb = min(ev, q0 + qn - s0)
                ps = psum.tile([128, ev, CO], f32, tag="ups", name="ups")
                get_ub(s0 // chb)
                get_ub((s0 + nb - 1) // chb)
                for qq in range(nb):
                    sl = s0 + qq
                    ub = ubs[sl // chb]
                    nc.tensor.matmul(
                        out=ps[:, qq, :], lhsT=ub[:, sl % chb, :],
                        rhs=wall[:, sl, :], start=True, stop=True)
                # (o,c)-ordered W columns -> straight PSUM->SBUF copy
                if ee is nc.vector:
                    nc.vector.tensor_copy(
                        out=uq[:, s0 - q0:s0 - q0 + nb, :, :]
                            .rearrange("p s o c -> p (s o c)"),
                        in_=ps[:, 0:nb, :].rearrange("p s x -> p (s x)"))
                else:
                    ee.copy(
                        uq[:, s0 - q0:s0 - q0 + nb, :, :]
                            .rearrange("p s o c -> p (s o c)"),
                        ps[:, 0:nb, :].rearrange("p s x -> p (s x)"))
                s0 += nb
        return quarters

    def route_core(g, uq, V_rep, s_ps, j, jlast):
        """One group's G/softmax/premul/s-matmul for one iteration.
        uq: list of uhat quarter tiles. The s-matmuls accumulate into the
        pair-shared s_ps [16, smm*CO] via the ones2[j] stationary."""
        # b-state scratch (bf16: G magnitudes are <<1)
        bst = state.tile([128, nslab, C], bf, tag="bst", name="bst")
        nchunk = nq

        # ---- G-step: bst = sum_o uhat * V_rep ----
        for ch in range(nchunk):
            sl = slice(ch * chs, (ch + 1) * chs)
            u4 = uq[ch]
            t2 = tmp.tile([128, chs, O, C], bf, tag="t2", bufs=3)
            nc.vector.tensor_tensor(
                out=t2[:], in0=u4[:],
                in1=bcast_ap(V_rep[:], 1, chs), op=Alu.mult)
            r1 = tmp.tile([128, chs, O // 2, C], bf, tag="r1t")
            nc.vector.tensor_tensor(
                out=r1[:], in0=t2[:, :, 0:O // 2, :],
                in1=t2[:, :, O // 2:O, :], op=Alu.add)
            r2 = tmp.tile([128, chs, O // 4, C], bf, tag="r2t")
            nc.vector.tensor_tensor(
                out=r2[:], in0=r1[:, :, 0:O // 4, :],
                in1=r1[:, :, O // 4:O // 2, :], op=Alu.add)
            r3 = tmp.tile([128, chs, 2, C], bf, tag="r3t")
            nc.vector.tensor_tensor(
                out=r3[:], in0=r2[:, :, 0:2, :],
                in1=r2[:, :, 2:4, :], op=Alu.add)
            nc.vector.tensor_tensor(
                out=bst[:, sl, :], in0=r3[:, :, 0, :],
                in1=r3[:, :, 1, :], op=Alu.add)
        # ---- softmax over c (2 coarse chunks to cut instr count) ----
        expt = tmp.tile([128, nslab, C], bf, tag="expt", bufs=2)
        Z = tmp.tile([128, nslab], f32, tag="Z", bufs=1)
        rz = tmp.tile([128, nslab], f32, tag="rz", bufs=1)
        cw = tmp.tile([128, nslab, C], bf, tag="cw", bufs=2)
        nsoft = min(2, nchunk)
        sft = nslab // nsoft
        for hf in range(nsoft):
            hs = slice(hf * sft, (hf + 1) * sft)
            nc.scalar.activation(expt[:, hs, :], bst[:, hs, :], Act.Exp)
            nc.vector.tensor_reduce(
                out=Z[:, hs], in_=expt[:, hs, :], axis=Ax.X, op=Alu.add)
            nc.vector.reciprocal(rz[:, hs], Z[:, hs])
            nc.vector.tensor_tensor(
                out=cw[:, hs, :], in0=expt[:, hs, :],
                in1=bcast_ap(rz[:, hs], 2, C), op=Alu.mult)
        # ---- s-step: premul + PE block-diag ones reduction ----
        for ch in range(nchunk):
            sl = slice(ch * chs, (ch + 1) * chs)
            u4 = uq[ch]
            t1 = tmp.tile([128, chs, O, C], bf, tag="t2", bufs=3)
            nc.vector.tensor_tensor(
                out=t1[:], in0=u4[:],
                in1=bcast_ap(cw[:, sl, :], 2, O), op=Alu.mult)
            for k in range(chs // smm):
                s_idx = ch * chs + k * smm
                nc.tensor.matmul(
                    out=s_ps[:], lhsT=ones_sb[:, j, :],
                    rhs=t1[:, k * smm:(k + 1) * smm, :, :],
                    start=(j == 0 and s_idx == 0),
                    stop=(j == jlast and s_idx == nslab - smm))

    def collect_s(s_ps, nrow):
        """Sum the smm slab-positions -> s_pair [nrow, CO] ((o,c) order)."""
        s_pair = small.tile([nrow, CO], f32, tag="s_pair", bufs=2)
        if smm == 2:
            s_rw = small.tile([nrow, 2 * CO], f32, tag="s_rw")
            nc.scalar.copy(s_rw[:], s_ps[0:nrow, :])
            nc.vector.tensor_tensor(
                out=s_pair[:], in0=s_rw[:, 0:CO], in1=s_rw[:, CO:2 * CO],
                op=Alu.add)
        else:
            nc.scalar.copy(s_pair[:], s_ps[0:nrow, 0:CO])
        return s_pair

    def emit_final(s_ps, nrow, row0):
        """Deferred final squash + output DMA for a pair."""
        s_pair = collect_s(s_ps, nrow)
        v_sb = squash(s_pair, nrow, ROUTING_ITERS - 1)
        nc.sync.dma_start(
            out=vout_dram[row0:row0 + nrow, :], in_=v_sb[:])

    v_bf1 = emit_s1()

    # Pair loop; the previous pair's FINAL squash is emitted after this
    # pair's it1 routing so it fills the squash/vrep stall.
    pending_final = None
    for j in range(npair):
        ga, gb = 2 * j, 2 * j + 1
        two = gb < ngroup
        nrow = 2 * BT if two else BT
        jlast = 1 if two else 0
        # first pair: evacuate ga's first quarter on the (idle) Vector
        # engine so routing can start as soon as that quarter lands
        ea = ([nc.vector] + [nc.scalar] * (nq - 1)) if j == 0 else None
        ua = phase_a(ga, evac_engs=ea)
        Va = state.tile([128, O, C], bf, tag="vrep", name="vrep", bufs=4)
        load_vrep(Va, v_bf1, ga * BT)
        Vb = None
        if two:
            Vb = state.tile([128, O, C], bf, tag="vrep", name="vrepb",
                            bufs=4)
            load_vrep(Vb, v_bf1, gb * BT)
        ub = phase_a(gb) if two else None
        for it in range(1, ROUTING_ITERS):
            s_ps = psum_acc.tile([2 * BT, smm * CO], f32, tag="sps",
                                 name="sps")
            route_core(ga, ua, Va, s_ps, 0, jlast)
            if two:
                route_core(gb, ub, Vb, s_ps, 1, jlast)
            if pending_final is not None:
                emit_final(*pending_final)
                pending_final = None
            last = (it == ROUTING_ITERS - 1)
            if last:
                pending_final = (s_ps, nrow, ga * BT)
            else:
                s_pair = collect_s(s_ps, nrow)
                v_bf = squash(s_pair, nrow, it, vk=v1k[j])
                Va = state.tile([128, O, C], bf, tag="vrep", name="vrep2",
                                bufs=4)
                load_vrep(Va, v_bf, 0)
                if two:
                    Vb = state.tile([128, O, C], bf, tag="vrep",
                                    name="vrep2b", bufs=4)
                    load_vrep(Vb, v_bf, BT)
    if pending_final is not None:
        emit_final(*pending_final)


def make_inputs_per_core(u, W):
    """Full inputs -> list of 8 in_maps."""
    W0 = np.asarray(W, dtype=np.float32)[0]
    u = np.asarray(u, dtype=np.float32)
    in_maps = []
    for c in range(NCORES):
        u_core = u[c * B_LOC:(c + 1) * B_LOC]
        in_maps.append(_host_prep(u_core, W0))
    return in_maps


def numpy_model(u_core, W0):
    """f32 numpy model of the routing (for small-scale checks)."""
    u_hat = np.einsum('pcoi,bpi->bpco', W0, u_core)
    b = np.zeros(u_hat.shape[:3], dtype=np.float32)
    v = None
    for _ in range(ROUTING_ITERS):
        e = np.exp(b - b.max(axis=2, keepdims=True))
        c = e / e.sum(axis=2, keepdims=True)
        s = np.einsum('bpc,bpco->bco', c, u_hat)
        sq = (s * s).sum(-1, keepdims=True)
        v = (sq / (1 + sq)) * s / np.sqrt(sq + EPS)
        b = b + np.einsum('bpco,bco->bpc', u_hat, v)
    return v


_COMPILED = {}


def _get_compiled():
    if "nc" in _COMPILED:
        return _COMPILED["nc"]
    from contextlib import ExitStack
    import concourse.tile as tile
    from concourse import bacc

    nc = bacc.Bacc("TRN2", target_bir_lowering=False, debug=False,
                   num_devices=NCORES)
    with tile.TileContext(nc) as tc:
        with ExitStack() as ctx:
            build(nc, tc, ctx)
    nc.compile()
    _COMPILED["nc"] = nc
    return nc


def kernel(u, W):
    """Full-input entry point: u [256,2048,8] f32, W [1,2048,10,16,8] f32
    -> v [256, 10, 16] f32."""
    from concourse.bass_utils import run_bass_kernel_spmd

    nc = _get_compiled()
    in_maps = make_inputs_per_core(u, W)
    res = run_bass_kernel_spmd(nc, in_maps, core_ids=list(range(NCORES)))
    outs = [res.results[c]["v_out"] for c in range(NCORES)]
    # v_out rows are (o,c)-major -> [B, O, C] -> transpose to [B, C, O]
    v = np.concatenate(outs, axis=0).reshape(B, O, C)
    return np.ascontiguousarray(v.transpose(0, 2, 1)).astype(np.float32)


# revision 64
# speedup vs baseline: 1.3792x; 1.0451x over previous
"""Trainium2 Bass kernel for DigitCapsules dynamic routing (v4).

Problem: u [256, 2048, 8] f32, W [1, 2048, 10, 16, 8] f32
  u_hat = einsum('pcoi,bpi->bpco', W[0], u)
  3 routing iterations (softmax over c, weighted sum over p, squash,
  agreement update) -> v [256, 10, 16] f32.

Strategy (8 cores, data-parallel over batch, 32 batch elems per core):
  - Partition layout: slabs of 16 p-values; SBUF partition index =
    (p_local * 8 + b_member); the PE contraction runs over
    K = (p_local 16, i 8) = 128 via a block-diagonal stationary u_bd
    (host-built; the zeros cost nothing at matmul time).
  - W columns are (o,c)-ordered host-side, so PSUM arrives as
    [slab, o, c] and evacuation is a straight (non-transposing) copy.
  - u_hat is materialized as FOUR quarter tiles per group ([128, 32, O,
    C] bf16, c innermost) so routing starts as soon as the first
    quarter lands instead of waiting for all 128 slabs.
  - Iteration 1 needs no u_hat read: s1 = 0.1 * sum_p u_hat from a
    dense-u stationary matmul accumulated over all slabs.
  - Iterations 2,3: G via DVE mul + add-tree over o; softmax over c via
    ACT exp + DVE reduce; weighted s-sum via PE matmuls with TWO
    block-diagonal ones stationaries that accumulate the group pair
    into one [16, 320] PSUM tile (squash is pair-batched).
  - Squash: one-Newton rsqrt (bit-trick seed), ~15 DVE ops per pair.
  - v never roundtrips through DRAM: V_rep replication and the v1
    accumulator are SBUF->SBUF block-copy DMAs.
  - The first pair's FINAL squash is emitted after the second pair's
    it1 routing so it fills the second pair's squash/vrep stall.
"""

import numpy as np
import ml_dtypes

bf16 = ml_dtypes.bfloat16

# Problem constants (fixed by the problem spec; do not read spec.json here)
B, P, C, O, IN = 256, 2048, 10, 16, 8
NCORES = 8
B_LOC = B // NCORES          # 32 batch elems per core
BT = 8                       # batch elems per group (one octet)
NGROUP = B_LOC // BT         # 4 groups per core
PSLAB = 16                   # p-values per slab
NSLAB = P // PSLAB           # 128 slabs
CO = C * O                   # 160
ROUTING_ITERS = 3
EPS = 1e-9

CHB = 8     # slabs per u_bd DMA chunk
EV = 3      # slabs per PSUM evacuation batch (1 bank per tile)
SMM = 2     # slabs per s-step matmul (N = SMM*CO = 320 <= 512)
CHS = 32    # slabs per routing compute chunk (= uhat quarter size)


def _host_prep(u_core, W0, nslab=NSLAB, ngroup=NGROUP):
    """Build host-side reordered (k-major, contiguous-DMA) arrays."""
    # w_k[p*8+i, s, o*10+c] = W0[16s+p, c, o, i]  ((o,c)-ordered columns)
    w = W0.reshape(nslab, PSLAB, C, O, IN)
    w_k = np.ascontiguousarray(
        w.transpose(1, 4, 0, 3, 2).reshape(PSLAB * IN, nslab, CO)
    ).astype(bf16)

    # x[g, b, s, p, i] = u_core[g*8 + b, 16s+p, i]
    x = u_core.reshape(ngroup, BT, nslab, PSLAB, IN)

    # ubd_k[g, ci, p*8+i, s_in_chunk, p'*8+b] = x[g,b,ci*chb+s,p,i]*(p==p')
    # -- chunk-major so every u_bd DMA is one fully contiguous read
    xt = x.transpose(0, 3, 4, 2, 1)  # [g, p, i, s, b]
    ubd_k = np.zeros((ngroup, PSLAB, IN, nslab, PSLAB, BT), dtype=bf16)
    for p in range(PSLAB):
        ubd_k[:, p, :, :, p, :] = xt[:, p]
    chb = min(CHB, nslab)
    ubd_k = np.ascontiguousarray(
        ubd_k.reshape(ngroup, PSLAB * IN, nslab // chb, chb, PSLAB * BT)
        .transpose(0, 2, 1, 3, 4))

    # ut_k[p*8+i, s, g*8+b] = x[g,b,s,p,i] -- one dense stationary for the
    # s1 matmul covering ALL groups (M = ngroup*BT)
    ut_k = np.ascontiguousarray(
        x.transpose(3, 4, 2, 0, 1).reshape(PSLAB * IN, nslab, ngroup * BT)
    ).astype(bf16)

    # ones2[j, p*8+b, j'*8+b'] = (b == b') & (j == j') -- stationaries for
    # the s-reduction; group j of a pair fills output rows j*8..j*8+8 while
    # contributing zeros to the other half (PSUM-accumulated jointly).
    ones2 = np.zeros((2, PSLAB * BT, 2 * BT), dtype=bf16)
    for j in range(2):
        for p in range(PSLAB):
            for b in range(BT):
                ones2[j, p * BT + b, j * BT + b] = 1.0
    return {
        "w_k": w_k,
        "ubd_k": ubd_k,
        "ut_k": ut_k,
        "ones2": ones2,
    }


def build(nc, tc, ctx, nslab=NSLAB, ngroup=NGROUP):
    """Emit the kernel IR. Parameterized slab/group counts for small tests."""
    import concourse.bass as bass
    from concourse import mybir

    f32 = mybir.dt.float32
    i32 = mybir.dt.int32
    bf = mybir.dt.bfloat16
    Alu = mybir.AluOpType
    Act = mybir.ActivationFunctionType
    Ax = mybir.AxisListType

    b_loc = ngroup * BT
    chb = min(CHB, nslab)
    ev = min(EV, nslab)
    smm = min(SMM, nslab)
    chs = min(CHS, nslab)
    nq = (nslab + chs - 1) // chs   # quarters per group

    # ---- DRAM parameters ----
    w_dram = nc.dram_tensor(
        "w_k", [PSLAB * IN, nslab, CO], bf, kind="ExternalInput").ap()
    ubd_dram = nc.dram_tensor(
        "ubd_k", [ngroup, nslab // min(CHB, nslab), PSLAB * IN,
                  min(CHB, nslab), PSLAB * BT], bf,
        kind="ExternalInput").ap()
    ut_dram = nc.dram_tensor(
        "ut_k", [PSLAB * IN, nslab, ngroup * BT], bf,
        kind="ExternalInput").ap()
    ones_dram = nc.dram_tensor(
        "ones2", [2, PSLAB * BT, 2 * BT], bf, kind="ExternalInput").ap()
    vout_dram = nc.dram_tensor(
        "v_out", [b_loc, CO], f32, kind="ExternalOutput").ap()
    vscr_dram = nc.dram_tensor("v_scratch", [b_loc, O * C], bf).ap()

    # ---- pools ----
    consts = ctx.enter_context(tc.tile_pool(name="consts", bufs=1))
    ubdpool = ctx.enter_context(tc.tile_pool(name="ubdpool", bufs=2))
    utpool = ctx.enter_context(tc.tile_pool(name="utpool", bufs=1))
    uhatpool = ctx.enter_context(tc.tile_pool(name="uhat", bufs=2 * nq))
    psum = ctx.enter_context(tc.tile_pool(name="psum", bufs=5, space="PSUM"))
    psum_acc = ctx.enter_context(
        tc.tile_pool(name="psum_acc", bufs=2, space="PSUM"))
    small = ctx.enter_context(tc.tile_pool(name="small", bufs=2))
    state = ctx.enter_context(tc.tile_pool(name="state", bufs=2))
    tmp = ctx.enter_context(tc.tile_pool(name="tmp", bufs=2))

    ones_sb = consts.tile([PSLAB * BT, 2, 2 * BT], bf)
    for j in range(2):
        nc.sync.dma_start(out=ones_sb[:, j, :], in_=ones_dram[j])
    magic = consts.tile([128, 1], i32)
    nc.gpsimd.memset(magic[:], 0x5F3759DF)


    # v1 accumulator per pair [16, O*C] (o,c)-major bf16
    npair = (ngroup + 1) // 2
    v1k = [consts.tile([2 * BT, O * C], bf, name=f"v1k{j}")
           for j in range(npair)]

    # resident W: whole tensor, ~256KB pieces round-robin over the queues
    # (each dma_start lands on one HW DMA engine at ~30GB/s; small pieces
    # spread the load so the first slabs arrive fast)
    wall = consts.tile([PSLAB * IN, nslab, CO], bf)

    def bcast_ap(ap, insert_pos, size):
        """Insert a stride-0 dim of `size` at free-dim position insert_pos."""
        new = list(ap.ap)
        new.insert(insert_pos, [0, size])
        return bass.AP(tensor=ap.tensor, offset=ap.offset, ap=new)

    def squash(s_sb, n, it, vk=None):
        """s_sb: [n, CO] f32, (o,c)-major. it < last: returns v_bf [n, O*C]
        bf16 (o,c) with accumulated V (vk + v) when vk given; else final:
        returns v_sb f32 (o,c).  factor = nrm/((1+nrm)*sqrt(nrm+eps)),
        rsqrt via bit-trick seed + one Newton step (~0.2% rel err)."""
        s3 = s_sb[:].rearrange("n (o c) -> n c o", o=O)
        sq = small.tile([n, CO], f32, tag="sqsq", bufs=1)
        nc.vector.tensor_mul(sq[:].rearrange("n (o c) -> n c o", o=O), s3, s3)
        nrm = small.tile([n, C], f32, tag="nrm")
        nc.vector.tensor_reduce(
            out=nrm[:], in_=sq[:].rearrange("n (o c) -> n c o", o=O),
            axis=Ax.X, op=Alu.add)
        # fac = nrm * rsqrt(q), q = nrm*(1+nrm)^2  (== sqrt(nrm)/(1+nrm),
        # the squash factor); no reciprocal needed, q=0 -> fac=0 cleanly.
        d1 = small.tile([n, C], f32, tag="d1")
        nc.vector.tensor_scalar_add(d1[:], nrm[:], 1.0)
        q = small.tile([n, C], f32, tag="qq")
        nc.vector.tensor_mul(q[:], nrm[:], d1[:])
        nc.vector.tensor_mul(q[:], q[:], d1[:])
        sh = small.tile([n, C], i32, tag="sh")
        nc.vector.tensor_scalar(
            out=sh[:], in0=q[:].bitcast(i32), scalar1=1, scalar2=None,
            op0=Alu.logical_shift_right)
        y0 = small.tile([n, C], i32, tag="y0")
        nc.vector.tensor_tensor(
            out=y0[:], in0=bcast_ap(magic[0:n, :], 1, C), in1=sh[:],
            op=Alu.subtract)
        y = y0[:].bitcast(f32)
        aa = small.tile([n, C], f32, tag="na")
        nc.vector.tensor_tensor(out=aa[:], in0=y, in1=y, op=Alu.mult)
        nc.vector.tensor_tensor(out=aa[:], in0=aa[:], in1=q[:],
                                op=Alu.mult)
        nc.vector.tensor_scalar(
            out=aa[:], in0=aa[:], scalar1=-0.5, scalar2=1.5,
            op0=Alu.mult, op1=Alu.add)
        yn = small.tile([n, C], f32, tag="ny")
        nc.vector.tensor_tensor(out=yn[:], in0=y, in1=aa[:], op=Alu.mult)
        fac = small.tile([n, C], f32, tag="fac")
        nc.vector.tensor_mul(fac[:], nrm[:], yn[:])
        v_sb = small.tile([n, CO], f32, tag="v_sb")
        nc.vector.tensor_tensor(
            out=v_sb[:].rearrange("n (o c) -> n c o", o=O),
            in0=s3, in1=bcast_ap(fac[:], 2, O), op=Alu.mult)
        if it == ROUTING_ITERS - 1:
            return v_sb
        v_bf = small.tile([n, O * C], bf, tag="v_bf")
        if vk is not None:
            # accumulated V = v1 + v2 so the next iteration's logits come
            # out of one linear G pass
            nc.vector.tensor_tensor(
                out=v_bf[:], in0=v_sb[:], in1=vk[0:n, :], op=Alu.add)
        else:
            nc.vector.tensor_copy(out=v_bf[:], in_=v_sb[:])
        return v_bf

    def vstore(v_bf, row0, n, engsel=None):
        """Write v rows to the DRAM bounce (one descriptor)."""
        eng = engsel if engsel is not None else nc.sync
        eng.dma_start(out=vscr_dram[row0:row0 + n, :], in_=v_bf[0:n, :])

    def load_vrep(V_rep, row0, engsel=None):
        """Replicate bounce rows [row0:row0+8] across the 16 p-positions
        with ONE DRAM->SBUF read (stride-0 outer dim on the DRAM side)."""
        srcap = bass.AP(
            tensor=vscr_dram.tensor,
            offset=vscr_dram.offset + row0 * O * C,
            ap=[[0, PSLAB], [O * C, BT], [1, O * C]])
        eng = engsel if engsel is not None else nc.gpsimd
        eng.dma_start(out=V_rep[:], in_=srcap)

    # ut first (s1 stationary) streamed through chunked tiles, then W,
    # round-robin over the rings
    engs3 = [nc.scalar, nc.sync, nc.gpsimd]
    hu = max(1, nslab // 4)
    ut_chunks = []
    for jj, j in enumerate(range(0, nslab, hu)):
        uc = utpool.tile([PSLAB * IN, hu, ngroup * BT], bf, tag="utres",
                         bufs=2)
        engs3[jj % 3].dma_start(
            out=uc[:], in_=ut_dram[:, j:j + hu, :])
        ut_chunks.append(uc)
    h = max(1, nslab // 16)
    for jj, j in enumerate(range(0, nslab, h)):
        engs3[jj % 3].dma_start(
            out=wall[:, j:j + h, :], in_=w_dram[:, j:j + h, :])

    def emit_s1():
        # ------- s1 sweep: one accumulation for ALL groups (M=32) -------
        s1_ps = psum_acc.tile([ngroup * BT, CO], f32, tag="s1ps",
                              name="s1ps", bufs=1)
        for s in range(nslab):
            nc.tensor.matmul(
                out=s1_ps[:], lhsT=ut_chunks[s // hu][:, s % hu, :],
                rhs=wall[:, s, :], start=(s == 0), stop=(s == nslab - 1))
        s1_sb = consts.tile([ngroup * BT, CO], f32)
        nc.scalar.mul(s1_sb[:], s1_ps[:], 1.0 / C)
        v_bf1 = squash(s1_sb, ngroup * BT, 0)
        for j in range(npair):
            r = min(2 * BT, b_loc - j * 2 * BT)
            nc.sync.dma_start(out=v1k[j][0:r, :],
                                in_=v_bf1[j * 2 * BT:j * 2 * BT + r, :])
        return v_bf1

    def phase_a(g, q_from=0, q_to=None, evac_engs=None):
        """u_hat materialization for quarters [q_from, q_to); returns the
        quarter tiles. evac_engs: per-quarter engine for the PSUM->SBUF
        evacuation copy (defaults to scalar)."""
        if q_to is None:
            q_to = nq
        quarters = []
        ubs = {}

        def get_ub(ci):
            if ci not in ubs:
                ub = ubdpool.tile([PSLAB * IN, chb, PSLAB * BT], bf,
                                  tag="ubd", name="ubd")
                eng = nc.gpsimd if ci % 2 else nc.sync
                eng.dma_start(out=ub[:], in_=ubd_dram[g, ci])
                ubs[ci] = ub
            return ubs[ci]

        for qi, q0 in enumerate(
                range(q_from * chs, min(q_to * chs, nslab), chs)):
            qn = min(chs, nslab - q0)
            uq = uhatpool.tile([128, chs, O, C], bf, tag="uhat",
                               name=f"uhat{g}_{q0}")
            quarters.append(uq)
            ee = (evac_engs[qi] if evac_engs is not None
                  else nc.scalar)
            s0 = q0
            while s0 < q0 + qn:
                nb = min(ev, q0 + qn - s0)
                ps = psum.tile([128, ev, CO], f32, tag="ups", name="ups")
                get_ub(s0 // chb)
                get_ub((s0 + nb - 1) // chb)
                for qq in range(nb):
                    sl = s0 + qq
                    ub = ubs[sl // chb]
                    nc.tensor.matmul(
                        out=ps[:, qq, :], lhsT=ub[:, sl % chb, :],
                        rhs=wall[:, sl, :], start=True, stop=True)
                # (o,c)-ordered W columns -> straight PSUM->SBUF copy
                if ee is nc.vector:
                    nc.vector.tensor_copy(
                        out=uq[:, s0 - q0:s0 - q0 + nb, :, :]
                            .rearrange("p s o c -> p (s o c)"),
                        in_=ps[:, 0:nb, :].rearrange("p s x -> p (s x)"))
                else:
                    ee.copy(
                        uq[:, s0 - q0:s0 - q0 + nb, :, :]
                            .rearrange("p s o c -> p (s o c)"),
                        ps[:, 0:nb, :].rearrange("p s x -> p (s x)"))
                s0 += nb
        return quarters

    def route_core(g, uq, V_rep, s_ps, j, jlast):
        """One group's G/softmax/premul/s-matmul for one iteration.
        uq: list of uhat quarter tiles. The s-matmuls accumulate into the
        pair-shared s_ps [16, smm*CO] via the ones2[j] stationary."""
        # b-state scratch (bf16: G magnitudes are <<1)
        bst = state.tile([128, nslab, C], bf, tag="bst", name="bst")
        nchunk = nq

        # ---- G-step: bst = sum_o uhat * V_rep ----
        for ch in range(nchunk):
            sl = slice(ch * chs, (ch + 1) * chs)
            u4 = uq[ch]
            t2 = tmp.tile([128, chs, O, C], bf, tag="t2", bufs=3)
            nc.vector.tensor_tensor(
                out=t2[:], in0=u4[:],
                in1=bcast_ap(V_rep[:], 1, chs), op=Alu.mult)
            r1 = tmp.tile([128, chs, O // 2, C], bf, tag="r1t")
            nc.vector.tensor_tensor(
                out=r1[:], in0=t2[:, :, 0:O // 2, :],
                in1=t2[:, :, O // 2:O, :], op=Alu.add)
            r2 = tmp.tile([128, chs, O // 4, C], bf, tag="r2t")
            nc.vector.tensor_tensor(
                out=r2[:], in0=r1[:, :, 0:O // 4, :],
                in1=r1[:, :, O // 4:O // 2, :], op=Alu.add)
            r3 = tmp.tile([128, chs, 2, C], bf, tag="r3t")
            nc.vector.tensor_tensor(
                out=r3[:], in0=r2[:, :, 0:2, :],
                in1=r2[:, :, 2:4, :], op=Alu.add)
            nc.vector.tensor_tensor(
                out=bst[:, sl, :], in0=r3[:, :, 0, :],
                in1=r3[:, :, 1, :], op=Alu.add)
        # ---- softmax over c (2 coarse chunks to cut instr count) ----
        expt = tmp.tile([128, nslab, C], bf, tag="expt", bufs=2)
        Z = tmp.tile([128, nslab], f32, tag="Z", bufs=1)
        rz = tmp.tile([128, nslab], f32, tag="rz", bufs=1)
        cw = tmp.tile([128, nslab, C], bf, tag="cw", bufs=2)
        nsoft = min(2, nchunk)
        sft = nslab // nsoft
        for hf in range(nsoft):
            hs = slice(hf * sft, (hf + 1) * sft)
            nc.scalar.activation(expt[:, hs, :], bst[:, hs, :], Act.Exp)
            nc.vector.tensor_reduce(
                out=Z[:, hs], in_=expt[:, hs, :], axis=Ax.X, op=Alu.add)
            nc.vector.reciprocal(rz[:, hs], Z[:, hs])
            nc.vector.tensor_tensor(
                out=cw[:, hs, :], in0=expt[:, hs, :],
                in1=bcast_ap(rz[:, hs], 2, C), op=Alu.mult)
        # ---- s-step: premul + PE block-diag ones reduction ----
        for ch in range(nchunk):
            sl = slice(ch * chs, (ch + 1) * chs)
            u4 = uq[ch]
            t1 = tmp.tile([128, chs, O, C], bf, tag="t2", bufs=3)
            nc.vector.tensor_tensor(
                out=t1[:], in0=u4[:],
                in1=bcast_ap(cw[:, sl, :], 2, O), op=Alu.mult)
            for k in range(chs // smm):
                s_idx = ch * chs + k * smm
                nc.tensor.matmul(
                    out=s_ps[:], lhsT=ones_sb[:, j, :],
                    rhs=t1[:, k * smm:(k + 1) * smm, :, :],
                    start=(j == 0 and s_idx == 0),
                    stop=(j == jlast and s_idx == nslab - smm))

    def collect_s(s_ps, nrow):
        """Sum the smm slab-positions -> s_pair [nrow, CO] ((o,c) order)."""
        s_pair = small.tile([nrow, CO], f32, tag="s_pair", bufs=2)
        if smm == 2:
            # copy+add both on the DVE (stalled at every boundary anyway);
            # a single PSUM source per instruction (one DVE PSUM port)
            s_rw = small.tile([nrow, 2 * CO], f32, tag="s_rw")
            nc.vector.tensor_copy(out=s_rw[:], in_=s_ps[0:nrow, :])
            nc.vector.tensor_tensor(
                out=s_pair[:], in0=s_rw[:, 0:CO], in1=s_rw[:, CO:2 * CO],
                op=Alu.add)
        else:
            nc.vector.tensor_copy(out=s_pair[:], in_=s_ps[0:nrow, 0:CO])
        return s_pair

    def emit_final(s_ps, nrow, row0):
        """Deferred final squash + output DMA for a pair."""
        s_pair = collect_s(s_ps, nrow)
        v_sb = squash(s_pair, nrow, ROUTING_ITERS - 1)
        nc.sync.dma_start(
            out=vout_dram[row0:row0 + nrow, :], in_=v_sb[:])

    v_bf1 = emit_s1()
    vstore(v_bf1, 0, b_loc)

    # Pair loop; the previous pair's FINAL squash is emitted after this
    # pair's it1 routing so it fills the squash/vrep stall.
    pending_final = None
    for j in range(npair):
        ga, gb = 2 * j, 2 * j + 1
        two = gb < ngroup
        nrow = 2 * BT if two else BT
        jlast = 1 if two else 0
        Va = Vb = None
        if j > 0:
            Va = state.tile([128, O, C], bf, tag="vrep", name="vrep",
                            bufs=4)
            load_vrep(Va, ga * BT)
            if two:
                Vb = state.tile([128, O, C], bf, tag="vrep", name="vrepb",
                                bufs=4)
                load_vrep(Vb, gb * BT)
        if j == 0:
            Va = state.tile([128, O, C], bf, tag="vrep", name="vrep",
                            bufs=4)
            load_vrep(Va, ga * BT, engsel=nc.gpsimd)
            if two:
                Vb = state.tile([128, O, C], bf, tag="vrep", name="vrepb",
                                bufs=4)
                load_vrep(Vb, gb * BT, engsel=nc.gpsimd)
        # first pair: evacuate ga's first quarter on the (idle) Vector
        # engine so routing can start as soon as that quarter lands
        ea = ([nc.vector] + [nc.scalar] * (nq - 1)) if j == 0 else None
        ua = phase_a(ga, evac_engs=ea)
        ub = phase_a(gb) if two else None
        for it in range(1, ROUTING_ITERS):
            s_ps = psum_acc.tile([2 * BT, smm * CO], f32, tag="sps",
                                 name="sps")
            route_core(ga, ua, Va, s_ps, 0, jlast)
            if two:
                route_core(gb, ub, Vb, s_ps, 1, jlast)
            if pending_final is not None:
                emit_final(*pending_final)
                pending_final = None
            last = (it == ROUTING_ITERS - 1)
            if last:
                pending_final = (s_ps, nrow, ga * BT)
            else:
                s_pair = collect_s(s_ps, nrow)
                v_bf = squash(s_pair, nrow, it, vk=v1k[j])
                vstore(v_bf, ga * BT, nrow)
                Va = state.tile([128, O, C], bf, tag="vrep", name="vrep2",
                                bufs=4)
                load_vrep(Va, ga * BT)
                if two:
                    Vb = state.tile([128, O, C], bf, tag="vrep",
                                    name="vrep2b", bufs=4)
                    load_vrep(Vb, gb * BT)
    if pending_final is not None:
        emit_final(*pending_final)


def make_inputs_per_core(u, W):
    """Full inputs -> list of 8 in_maps."""
    W0 = np.asarray(W, dtype=np.float32)[0]
    u = np.asarray(u, dtype=np.float32)
    in_maps = []
    for c in range(NCORES):
        u_core = u[c * B_LOC:(c + 1) * B_LOC]
        in_maps.append(_host_prep(u_core, W0))
    return in_maps


def numpy_model(u_core, W0):
    """f32 numpy model of the routing (for small-scale checks)."""
    u_hat = np.einsum('pcoi,bpi->bpco', W0, u_core)
    b = np.zeros(u_hat.shape[:3], dtype=np.float32)
    v = None
    for _ in range(ROUTING_ITERS):
        e = np.exp(b - b.max(axis=2, keepdims=True))
        c = e / e.sum(axis=2, keepdims=True)
        s = np.einsum('bpc,bpco->bco', c, u_hat)
        sq = (s * s).sum(-1, keepdims=True)
        v = (sq / (1 + sq)) * s / np.sqrt(sq + EPS)
        b = b + np.einsum('bpco,bco->bpc', u_hat, v)
    return v


_COMPILED = {}


def _get_compiled():
    if "nc" in _COMPILED:
        return _COMPILED["nc"]
    from contextlib import ExitStack
    import concourse.tile as tile
    from concourse import bacc

    nc = bacc.Bacc("TRN2", target_bir_lowering=False, debug=False,
                   num_devices=NCORES)
    with tile.TileContext(nc) as tc:
        with ExitStack() as ctx:
            build(nc, tc, ctx)
    nc.compile()
    _COMPILED["nc"] = nc
    return nc


def kernel(u, W):
    """Full-input entry point: u [256,2048,8] f32, W [1,2048,10,16,8] f32
    -> v [256, 10, 16] f32."""
    from concourse.bass_utils import run_bass_kernel_spmd

    nc = _get_compiled()
    in_maps = make_inputs_per_core(u, W)
    res = run_bass_kernel_spmd(nc, in_maps, core_ids=list(range(NCORES)))
    outs = [res.results[c]["v_out"] for c in range(NCORES)]
    # v_out rows are (o,c)-major -> [B, O, C] -> transpose to [B, C, O]
    v = np.concatenate(outs, axis=0).reshape(B, O, C)
    return np.ascontiguousarray(v.transpose(0, 2, 1)).astype(np.float32)
